# revision 1
# baseline (speedup 1.0000x reference)
"""MoE (top-2 of 8 experts + shared expert) Trainium2 kernel, expert-parallel
across 8 NeuronCores.

Strategy:
  - Host: compute the (tiny) gate in float64 numpy, select top-2 experts per
    token, and dispatch tokens by routing index (the all-to-all of
    expert-parallel MoE, done during the host-side shard step).
  - Work is balanced by slot packing: every core runs 3 routed fixed-capacity
    token slots plus one shared-expert slot of 512 tokens.  The slot caps are
    chosen at runtime by a small search that minimizes total capacity
    (8*sum(caps) >= routed tokens) subject to an exact-cover feasibility DP,
    so padding waste is ~1-2% instead of the 25% a fixed two-cap scheme costs.
  - Device (per core): feature-major MLP per slot, all operands bf16 (full
    PE rate, half the DMA traffic of fp32).  x^T and h stay resident in SBUF;
    weights stream it-tile by it-tile as one fused DMA per i-tile (w1e|w3e|
    w1o|w3o concatenated -> 8KB contiguous lines); swiglu is 5 DVE + 3 ACT
    ops per tile (Silu activation fuses sigmoid*mul); second GEMM accumulates
    over 16 i-tiles and writes bf16 y.
  - Scheduling: slots run largest-cap first (best PE-work-per-weight-byte
    while the DMA pipeline is cold); the next slot's x/bias loads trigger
    from the GpSimd queue and y writebacks from the Scalar queue so the
    in-order Sync queue only carries weight loads (no head-of-line blocking
    behind y-ready semaphores); 5 weight i-tiles of the next slot prefetch
    ahead of each slot's second GEMM.  Measured ~445-465us vs the 598us
    fp32r two-slot baseline (PE busy ~423us vs a ~414us matmul-row floor,
    MFU ~91%; run-to-run spread above that is device power throttling).
  - Host: combine = scatter-add of per-piece outputs weighted by the gate
    probabilities (1.0 for shared slices).  The swiglu even/odd interleave
    split, transposes, and the 1/1.702 silu rescale are pre-folded into the
    host-side weight layouts.
"""
import sys

sys.path.insert(0, "/opt/trn_rl_repo")

import itertools

import ml_dtypes
import numpy as np

import concourse.bacc as bacc_mod
import concourse.tile as tile
from concourse import mybir
from concourse.bass_utils import run_bass_kernel_spmd

F32 = mybir.dt.float32
BF16 = mybir.dt.bfloat16
NP_BF16 = ml_dtypes.bfloat16
Alu = mybir.AluOpType
Act = mybir.ActivationFunctionType

ALPHA = 1.702
LIMIT = 7.0
TOPK = 2
D, I, E = 1024, 2048, 8
B, S = 2, 2048
T = B * S
DK = D // 128          # 8 d-tiles
IT = I // 128          # 16 i-tiles
TS = 512               # shared-expert tokens per core (T / 8)
N_CORES = 8
NB = 4 * IT + DK       # bias-pack columns

_kernel_cache = {}


# --------------------------------------------------------------------------
# slot-cap planning: minimize total per-core routed capacity subject to an
# exact-cover feasibility DP (each cap has 8 instances, one per core).
# --------------------------------------------------------------------------

def _cover(caps, counts, ninst=8):
    """Assign instances of each cap to experts covering counts.
    Returns per-expert tuples n_ej or None if infeasible."""
    k = len(caps)
    per = []
    for cnt in counts:
        out = []
        for combo in itertools.product(range(ninst + 1), repeat=k):
            cap = sum(n * c for n, c in zip(combo, caps))
            if cap >= cnt:
                out.append((cap - cnt, combo))
        if not out:
            return None
        out.sort()
        per.append([c for _, c in out[:64]])
    memo = {}

    def dp(i, used):
        if i == len(counts):
            return []
        key = (i, used)
        if key in memo:
            return memo[key]
        res = None
        for combo in per[i]:
            nu = tuple(u + n for u, n in zip(used, combo))
            if any(u > ninst for u in nu):
                continue
            sub = dp(i + 1, nu)
            if sub is not None:
                res = [combo] + sub
                break
        memo[key] = res
        return res

    return dp(0, (0,) * k)


def _caps_at(C, counts, step):
    """All feasible (caps, asg) at capacity C, preferring a large min cap."""
    best = None
    for c1 in range(min(512, C - 384), 191, -step):
        for c2 in range(min(c1, C - c1 - 192), 191, -step):
            c3 = C - c1 - c2
            if c3 < 192 or c3 > c2:
                continue
            if best is not None and c3 <= best[0][2]:
                continue
            asg = _cover([c1, c2, c3], counts)
            if asg is not None:
                best = ([c1, c2, c3], asg)
    return best


# near-optimal triples found by offline fine-grained sweeps; _cover
# validates them against the actual counts, so they only apply when legal
_PROBES = ((432, 324, 278), (432, 324, 280), (424, 332, 280),
           (428, 328, 280), (432, 320, 284), (424, 328, 284))


def _plan_caps(counts):
    """Pick routed slot caps (each in [192, 512]) minimizing per-core
    capacity, then preferring balanced caps. Returns (caps, assignment)."""
    for C in range(1024, 1296, 16):
        best = _caps_at(C, counts, 16)
        if best is not None:
            # cheap fine-grained probes that beat the step-16 grid
            for probe in _PROBES:
                if sum(probe) < C:
                    asg = _cover(list(probe), counts)
                    if asg is not None:
                        return list(probe), asg
            return best
    # robust fallback: always feasible (capacity 12288 >= 8192, and any
    # expert count <= 4096 = 8*512 spreads over equal cap positions)
    caps = [512, 512, 512]
    asg = _cover(caps, counts)
    if asg is None:
        caps = [512, 512, 512, 512]
        asg = _cover(caps, counts)
    return caps, asg


# --------------------------------------------------------------------------
# host-side packing to device layouts (all bf16 except biases)
# --------------------------------------------------------------------------

def _tile13(w):
    """[D, I] -> [IT, 128(k), DK, 128(m)]  (k = d%128, m = i%128)."""
    return w.reshape(DK, 128, IT, 128).transpose(2, 1, 0, 3)


def _expert_pack(w1, b1, w3, b3, w2, b2):
    wf = np.stack([_tile13(w1[:, 0::2]), _tile13(w3[:, 0::2]),
                   _tile13(w1[:, 1::2]), _tile13(w3[:, 1::2])], axis=2)
    wf = np.ascontiguousarray(wf.reshape(IT, 128, 4 * DK * 128))
    w2t = (w2 * np.float32(1.0 / ALPHA)).reshape(IT, 128, DK, 128)
    w2t = np.ascontiguousarray(w2t.transpose(2, 1, 0, 3).reshape(DK, 128, IT * 128))
    bias = np.concatenate([
        b1[0::2].reshape(IT, 128).T, b3[0::2].reshape(IT, 128).T,
        b1[1::2].reshape(IT, 128).T, b3[1::2].reshape(IT, 128).T,
        b2.reshape(DK, 128).T,
    ], axis=1)
    return {
        "wf": wf.astype(NP_BF16),
        "w2": w2t.astype(NP_BF16),
        "bias": np.ascontiguousarray(bias, dtype=np.float32),
    }


def _xt_pack(xsub, cap):
    """[n, D] tokens -> zero-padded [128, DK*cap] bf16 transposed layout."""
    n = xsub.shape[0]
    xt = np.zeros((D, cap), dtype=np.float32)
    xt[:, :n] = xsub.T
    xt = xt.reshape(DK, 128, cap).transpose(1, 0, 2)
    return np.ascontiguousarray(xt.reshape(128, DK * cap)).astype(NP_BF16)


# --------------------------------------------------------------------------
# device kernel
# --------------------------------------------------------------------------

def _groups(cap):
    gs = [512] * (cap // 512)
    if cap % 512:
        gs.append(cap % 512)
    offs = np.cumsum([0] + gs)[:-1]
    return list(zip(offs, gs))


def _build(caps):
    """Build the SPMD Bass kernel; caps = routed slot caps + [TS] shared."""
    nc = bacc_mod.Bacc("TRN2")

    def dram(name, shape, dtype, out=False):
        return nc.declare_dram_parameter(name, list(shape), dtype, isOutput=out)

    slots = []
    for j, cap in enumerate(caps):
        p = f"s{j}"
        w = {
            "xt": dram(p + "xt", [128, DK * cap], BF16),
            "wf": dram(p + "wf", [IT, 128, 4 * DK * 128], BF16),
            "w2": dram(p + "w2", [DK, 128, IT * 128], BF16),
            "bias": dram(p + "bias", [128, NB], F32),
            "y": dram(p + "y", [DK, 128, cap], BF16, out=True),
        }
        slots.append((j, cap, w))

    with tile.TileContext(nc) as tc:
        with (
            tc.tile_pool(name="persist", bufs=1) as persist,
            tc.tile_pool(name="wpool", bufs=6) as wpool,
            tc.tile_pool(name="w2pool", bufs=6) as w2pool,
            tc.tile_pool(name="work", bufs=2) as work,
            tc.tile_pool(name="outp", bufs=3) as outp,
            tc.tile_pool(name="ps", bufs=1, space="PSUM") as ps,
            tc.tile_pool(name="psy", bufs=3, space="PSUM") as psy,
        ):
            # persistent per-slot tiles, allocated upfront

            xts_t, bt_t, hb_t = {}, {}, {}
            for j, cap, w in slots:
                xts_t[j] = persist.tile([128, DK * cap], BF16, tag=f"xt{j}",
                                        name=f"xt_s{j}")
                bt_t[j] = persist.tile([128, NB], F32, tag=f"bias{j}",
                                       name=f"bias_s{j}")
                hb_t[j] = persist.tile([128, IT * cap], BF16, tag=f"h{j}",
                                       name=f"h_s{j}")

            def load_xt_bias(j):
                # gpsimd-queue triggers: keeps these off the Sync queue so
                # weight-load triggers are never stuck behind them
                _, cap, w = slots[j]
                if j == 0:
                    # split halves so the first matmuls wait on less data
                    half = DK * cap // 2
                    xap = w["xt"].ap()
                    nc.gpsimd.dma_start(out=xts_t[j][:, :half],
                                        in_=xap[:, :half])
                    nc.gpsimd.dma_start(out=xts_t[j][:, half:],
                                        in_=xap[:, half:])
                else:
                    nc.gpsimd.dma_start(out=xts_t[j], in_=w["xt"].ap())
                nc.gpsimd.dma_start(out=bt_t[j], in_=w["bias"].ap())

            load_xt_bias(0)

            def wf_load(j, it):
                _, _, w = slots[j]
                wt = wpool.tile([128, 4 * DK * 128], BF16, tag="wf",
                                name=f"wf_s{j}_{it}")
                if j == 0 and it <= 2:
                    # split per-w during pipeline fill so matmuls start on
                    # partial data instead of a full 1MB tile
                    for wi in range(4):
                        o = wi * DK * 128
                        nc.sync.dma_start(out=wt[:, o:o + DK * 128],
                                          in_=w["wf"][it][:, o:o + DK * 128])
                else:
                    nc.sync.dma_start(out=wt, in_=w["wf"][it])
                return wt

            wf_pre = {}
            for j, cap, w in slots:
                grp = _groups(cap)
                xts, bt, hb = xts_t[j], bt_t[j], hb_t[j]
                w2_pre = {}

                def prefetch_w2(dk):
                    w2t = w2pool.tile([128, IT * 128], BF16, tag="w2",
                                      name=f"w2_s{j}_{dk}")
                    nc.sync.dma_start(out=w2t, in_=w["w2"][dk])
                    w2_pre[dk] = w2t

                # ---- first GEMM + swiglu: h[it, tok] ----
                for it in range(IT):
                    wt = wf_pre.pop((j, it), None)
                    if wt is None:
                        wt = wf_load(j, it)
                    if it == 8 and j + 1 < len(slots):
                        load_xt_bias(j + 1)          # prefetch next slot x
                    if it in (5, 7, 9, 11, 13, 15):
                        prefetch_w2((it - 5) // 2)   # prefetch w2 head
                    for goff, gsz in grp:
                        accs = []
                        for wi in range(4):
                            acc = ps.tile([128, 512], F32, tag=f"acc{wi}",
                                          name=f"acc{wi}_s{j}_{it}_{goff}")
                            for dk in range(DK):
                                o = (wi * DK + dk) * 128
                                nc.tensor.matmul(
                                    acc[:, :gsz],
                                    wt[:, o:o + 128],
                                    xts[:, dk * cap + goff:
                                        dk * cap + goff + gsz],
                                    start=(dk == 0), stop=(dk == DK - 1))
                            accs.append(acc)
                        A, Bm, C, Dm = accs
                        Bp = work.tile([128, 512], F32, tag="Bp")
                        nc.scalar.activation(Bp[:, :gsz], Bm[:, :gsz],
                                             Act.Identity,
                                             bias=bt[:, IT + it:IT + it + 1])
                        G = work.tile([128, 512], F32, tag="G")
                        nc.vector.scalar_tensor_tensor(
                            G[:, :gsz], A[:, :gsz], bt[:, it:it + 1],
                            Bp[:, :gsz], Alu.add, Alu.mult)
                        nc.vector.tensor_scalar_min(G[:, :gsz], G[:, :gsz],
                                                    LIMIT)
                        # Sv = silu(alpha*G); the 1/alpha rescale is folded
                        # into w2 on the host
                        Sv = work.tile([128, 512], F32, tag="Sv")
                        nc.scalar.activation(Sv[:, :gsz], G[:, :gsz],
                                             Act.Silu, scale=ALPHA)
                        Dp = work.tile([128, 512], F32, tag="Dp")
                        nc.scalar.activation(
                            Dp[:, :gsz], Dm[:, :gsz], Act.Identity,
                            bias=bt[:, 3 * IT + it:3 * IT + it + 1])
                        L = work.tile([128, 512], F32, tag="L")
                        nc.vector.scalar_tensor_tensor(
                            L[:, :gsz], C[:, :gsz],
                            bt[:, 2 * IT + it:2 * IT + it + 1],
                            Dp[:, :gsz], Alu.add, Alu.mult)
                        nc.vector.tensor_scalar(L[:, :gsz], L[:, :gsz],
                                                LIMIT, -LIMIT,
                                                Alu.min, Alu.max)
                        nc.vector.scalar_tensor_tensor(
                            hb[:, it * cap + goff:it * cap + goff + gsz],
                            L[:, :gsz], 1.0, Sv[:, :gsz], Alu.add, Alu.mult)

                # prefetch the next slot's first weight tiles ahead of the
                # GEMM2 y-writeback triggers (Sync queue is in-order)
                if j + 1 < len(slots):
                    for it2 in range(5):
                        wf_pre[(j + 1, it2)] = wf_load(j + 1, it2)

                # ---- second GEMM: y[dk] = sum_it w2[dk,it].T @ h[it] ----
                for dk in range(DK):
                    if dk in w2_pre:
                        w2t = w2_pre.pop(dk)
                    else:
                        w2t = w2pool.tile([128, IT * 128], BF16, tag="w2",
                                          name=f"w2_s{j}_{dk}")
                        nc.sync.dma_start(out=w2t, in_=w["w2"][dk])
                    for goff, gsz in grp:
                        Y = psy.tile([128, 512], F32, tag="Y",
                                     name=f"Y_s{j}_{dk}_{goff}")
                        for it in range(IT):
                            nc.tensor.matmul(
                                Y[:, :gsz],
                                w2t[:, it * 128:(it + 1) * 128],
                                hb[:, it * cap + goff:it * cap + goff + gsz],
                                start=(it == 0), stop=(it == IT - 1))
                        yo = outp.tile([128, 512], BF16, tag="yo")
                        nc.scalar.activation(
                            yo[:, :gsz], Y[:, :gsz], Act.Identity,
                            bias=bt[:, 4 * IT + dk:4 * IT + dk + 1])
                        # scalar-queue trigger: fires right after the ACT
                        # above with no semaphore wait, and keeps y
                        # writebacks from head-of-line blocking Sync
                        nc.scalar.dma_start(
                            out=w["y"][dk, :, goff:goff + gsz],
                            in_=yo[:, :gsz])

    nc.finalize()
    return nc


# --------------------------------------------------------------------------
# entry point
# --------------------------------------------------------------------------

def kernel(x, gate_w, gate_b, w1, b1, w3, b3, w2, b2,
           sw1, sb1, sw3, sb3, sw2, sb2):
    x = np.asarray(x, dtype=np.float32)
    xt = x.reshape(T, D)

    # ---- gate (float64 host math; selection + combine weights) ----
    z = xt.astype(np.float64) @ np.asarray(gate_w, dtype=np.float64).T
    z -= z.max(axis=-1, keepdims=True)
    ez = np.exp(z)
    scores = ez / ez.sum(axis=-1, keepdims=True)          # [T, E]
    biased = scores + np.asarray(gate_b, dtype=np.float64)
    top2 = np.argsort(-biased, axis=-1, kind="stable")[:, :TOPK]   # [T, 2]
    gate_wt = np.take_along_axis(scores, top2, axis=-1).astype(np.float32)

    tok_idx = []
    tok_wt = []
    for e in range(E):
        sel = np.nonzero((top2 == e).any(axis=1))[0]
        we = np.where(top2[sel, 0] == e, gate_wt[sel, 0], gate_wt[sel, 1])
        tok_idx.append(sel)
        tok_wt.append(we.astype(np.float32))
    counts = [len(s) for s in tok_idx]

    # ---- plan slot caps + cut expert token lists into per-slot pieces ----
    rcaps, assign = _plan_caps(counts)
    k = len(rcaps)
    pieces = {j: [] for j in range(k)}       # slot idx -> list of (e, lo, hi)
    for e in range(E):
        lo = 0
        for j in range(k):
            for _ in range(assign[e][j]):
                hi = min(lo + rcaps[j], counts[e])
                pieces[j].append((e, lo, hi))
                lo = hi
        assert lo >= counts[e]
    for j in range(k):
        while len(pieces[j]) < N_CORES:
            pieces[j].append((0, 0, 0))

    # ---- build per-core input maps ----
    epacks = [
        _expert_pack(np.asarray(w1[e]), np.asarray(b1[e]),
                     np.asarray(w3[e]), np.asarray(b3[e]),
                     np.asarray(w2[e]), np.asarray(b2[e]))
        for e in range(E)
    ]
    spack = _expert_pack(np.asarray(sw1), np.asarray(sb1),
                         np.asarray(sw3), np.asarray(sb3),
                         np.asarray(sw2), np.asarray(sb2))
    slot_kinds = sorted([(rcaps[j], j) for j in range(k)] + [(TS, -1)],
                        key=lambda t: -t[0])        # largest cap first
    caps = tuple(cap for cap, _ in slot_kinds)
    in_maps = []
    for c in range(N_CORES):
        m = {}
        for s, (cap, kidx) in enumerate(slot_kinds):
            if kidx < 0:
                m[f"s{s}xt"] = _xt_pack(xt[c * TS:(c + 1) * TS], TS)
                pk = spack
            else:
                e, lo, hi = pieces[kidx][c]
                m[f"s{s}xt"] = _xt_pack(xt[tok_idx[e][lo:hi]], cap)
                pk = epacks[e]
            for kk, v in pk.items():
                m[f"s{s}{kk}"] = v
        in_maps.append(m)

    # ---- compile (cached) + run on all 8 cores ----
    if caps not in _kernel_cache:
        _kernel_cache[caps] = _build(caps)
    nc = _kernel_cache[caps]
    res = run_bass_kernel_spmd(nc, in_maps, list(range(N_CORES)))

    # ---- combine: weighted scatter-add of routed pieces + shared slices ----
    out = np.zeros((T, D), dtype=np.float32)
    for c in range(N_CORES):
        for s, (cap, kidx) in enumerate(slot_kinds):
            yc = res.results[c][f"s{s}y"].astype(np.float32).reshape(D, cap)
            if kidx < 0:
                out[c * TS:(c + 1) * TS] += yc.T
            else:
                e, lo, hi = pieces[kidx][c]
                if hi <= lo:
                    continue
                idx = tok_idx[e][lo:hi]
                out[idx] += tok_wt[e][lo:hi][:, None] * yc.T[:hi - lo]
    return out.reshape(B, S, D)



# revision 2
# speedup vs baseline: 1.1093x; 1.1093x over previous
"""MoE (top-2 of 8 experts + shared expert) Trainium2 kernel, expert-parallel
across 8 NeuronCores, hybrid fp16/fp8 precision.

Strategy (upgrade over the 445us bf16 baseline):
  - Host: gate in float64 numpy; tokens dispatched by routing index.
  - Precision hybrid: each routed expert-visit's error contribution to the
    final output is attenuated by its gate combine weight (mean ~0.24), while
    the shared expert enters with weight 1.  So the lowest-weight ~2/3 of
    routed visits (chosen by a global sum-w^2 error budget FRAC) run fully in
    fp8 e4m3 with DoubleRow matmuls (2x PE rate, measured 1.92x), and the
    high-weight rest + the shared expert run in fp16 (same PE rate as bf16,
    4 more mantissa bits -> smaller base error).
  - Quantization scales (w x32, x x8 for fp8, h x2) keep values out of fp8/
    fp16 denormals and are folded into ACT scale/bias constants -- zero extra
    device ops vs the baseline swiglu (5 DVE + 3 ACT per i-tile).
  - Per-core slots: [shared 512 f16] + p16 routed slot(s) (caps from a DP
    cover of the residual per-expert counts) + one fp8 slot (cap CF, one
    expert per core, so its weights stream once).  Planner minimizes
    sum(caps16) + 0.52*CF subject to the error budget.
  - Scheduling: largest-compute slot first; x/bias loads on the GpSimd queue,
    y writebacks on the Scalar queue, weight loads on Sync; next-slot x at
    it==8, 5 wf tiles prefetched ahead of each slot's GEMM2, w2 prefetched
    at odd i-tiles.
"""
import sys

sys.path.insert(0, "/opt/trn_rl_repo")

import itertools
import os

import ml_dtypes
import numpy as np

import concourse.bacc as bacc_mod
import concourse.tile as tile
from concourse import mybir
from concourse.bass_utils import run_bass_kernel_spmd

F32 = mybir.dt.float32
FP16 = mybir.dt.float16
FP8 = mybir.dt.float8e4
NP_F8 = ml_dtypes.float8_e4m3
Alu = mybir.AluOpType
Act = mybir.ActivationFunctionType

ALPHA = 1.702
LIMIT = 7.0
TOPK = 2
D, I, E = 1024, 2048, 8
B, S = 2, 2048
T = B * S
DK = D // 128          # 8 d-tiles
IT = I // 128          # 16 i-tiles
TS = 512               # shared-expert tokens per core (T / 8)
N_CORES = 8
NB = 4 * IT + DK       # bias-pack columns

# fraction of routed sum-w^2 allowed into fp8 (error budget)
FRAC = float(os.environ.get("MOE_FRAC", "0.42"))
FP8_COST = 0.52        # measured fp8 PE cost per token vs fp16

# per-class constants: s1 = SX*SW is the GEMM1 psum scale
CLS = {
    "p16": dict(dt=FP16, npdt=np.float16, SX=1.0, SW=32.0, SH=1.0, SW2=32.0,
                dbl=False),
    "p8": dict(dt=FP8, npdt=NP_F8, SX=8.0, SW=32.0, SH=2.0, SW2=32.0,
               dbl=True),
}

_kernel_cache = {}


# --------------------------------------------------------------------------
# host-side packing
# --------------------------------------------------------------------------

def _q(a, cls):
    if cls == "p8":
        return np.clip(a, -240.0, 240.0).astype(NP_F8)
    return a.astype(np.float16)


def _tile13(w):
    """[D, I] -> [IT, 128(k), DK, 128(m)]."""
    return w.reshape(DK, 128, IT, 128).transpose(2, 1, 0, 3)


def _expert_pack(w1, b1, w3, b3, w2, b2, cls):
    c = CLS[cls]
    s1 = c["SX"] * c["SW"]
    wf = np.stack([_tile13(w1[:, 0::2]), _tile13(w3[:, 0::2]),
                   _tile13(w1[:, 1::2]), _tile13(w3[:, 1::2])], axis=2)
    wf = np.ascontiguousarray(wf.reshape(IT, 128, 4 * DK, 128)) * c["SW"]
    w2t = (w2 * (c["SW2"] / ALPHA)).reshape(IT, 128, DK, 128)
    w2t = np.ascontiguousarray(w2t.transpose(2, 1, 0, 3))  # [DK,128,IT,128]
    bias = np.concatenate([
        s1 * b1[0::2].reshape(IT, 128).T,
        b3[0::2].reshape(IT, 128).T,
        s1 * b1[1::2].reshape(IT, 128).T,
        (c["SH"] / s1) * b3[1::2].reshape(IT, 128).T,
        b2.reshape(DK, 128).T,
    ], axis=1)
    return {
        "wf": _q(wf, cls),
        "w2": _q(w2t, cls),
        "bias": np.ascontiguousarray(bias, dtype=np.float32),
    }


def _xt_pack(xsub, cap, cls):
    """[n, D] tokens -> zero-padded [128, DK, cap] transposed layout."""
    c = CLS[cls]
    n = xsub.shape[0]
    xt = np.zeros((D, cap), dtype=np.float32)
    xt[:, :n] = (c["SX"] * xsub).T
    xt = np.ascontiguousarray(xt.reshape(DK, 128, cap).transpose(1, 0, 2))
    return _q(xt, cls)


# --------------------------------------------------------------------------
# planning
# --------------------------------------------------------------------------

def _cover(caps, counts, ninst=8):
    """Assign instances of each cap to experts covering counts."""
    k = len(caps)
    per = []
    for cnt in counts:
        out = []
        for combo in itertools.product(range(ninst + 1), repeat=k):
            cap = sum(n * c for n, c in zip(combo, caps))
            if cap >= cnt:
                out.append((cap - cnt, combo))
        if not out:
            return None
        out.sort()
        per.append([c for _, c in out[:64]])
    memo = {}

    def dp(i, used):
        if i == len(counts):
            return []
        key = (i, used)
        if key in memo:
            return memo[key]
        res = None
        for combo in per[i]:
            nu = tuple(u + n for u, n in zip(used, combo))
            if any(u > ninst for u in nu):
                continue
            sub = dp(i + 1, nu)
            if sub is not None:
                res = [combo] + sub
                break
        memo[key] = res
        return res

    return dp(0, (0,) * k)


def _plan_p16(counts):
    """Min-capacity cover of counts with 1-2 cap sizes (8 instances each)."""
    best = None
    mx = max(max(counts), 1)
    tot = sum(counts)
    lo = max(64, -(-tot // 8))
    # single cap
    for c in range(-(-max(mx, lo) // 16) * 16, -(-max(mx, lo) // 16) * 16 + 257, 16):
        asg = _cover([c], counts)
        if asg is not None:
            best = (c, [c], asg)
            break
    # cap pairs
    top = best[0] if best else 2 * mx
    for c1 in range(-(-lo // 2 // 16) * 16, min(515, top), 16):
        for c2 in range(64, c1 + 1, 16):
            if c1 + c2 >= top:
                continue
            asg = _cover([c1, c2], counts)
            if asg is not None:
                top = c1 + c2
                best = (c1 + c2, [c1, c2], asg)
    if best is None:
        return None
    return best[1], best[2]


def _plan_hybrid(n_e, wt_sorted):
    """Choose fp8 cap CF + per-expert fp8 takes + p16 caps/assignment.

    wt_sorted[e]: visit weights ascending.  Returns (CF, take8, caps16,
    asg16) minimizing caps16-total + FP8_COST*CF under the FRAC budget.
    """
    allw2 = np.concatenate([w ** 2 for w in wt_sorted])
    budget = FRAC * allw2.sum()
    cum = [np.concatenate([[0.0], np.cumsum(w ** 2)]) for w in wt_sorted]

    best = None
    for CF in range(192, 1025, 32):
        take = [min(n_e[e], CF) for e in range(E)]
        # enforce global budget: drop highest-weight tentative fp8 visits
        over = sum(cum[e][take[e]] for e in range(E)) - budget
        if over > 0:
            heads = []
            for e in range(E):
                heads.append((e, take[e]))
            # repeatedly remove the globally largest fp8 weight
            import heapq
            heap = [(-wt_sorted[e][t - 1], e) for e, t in heads if t > 0]
            heapq.heapify(heap)
            while over > 0 and heap:
                w2v, e = heapq.heappop(heap)
                over -= wt_sorted[e][take[e] - 1] ** 2
                take[e] -= 1
                if take[e] > 0:
                    heapq.heappush(heap, (-wt_sorted[e][take[e] - 1], e))
        resid = [n_e[e] - take[e] for e in range(E)]
        p16 = _plan_p16(resid)
        if p16 is None:
            continue
        caps16, asg16 = p16
        cost = sum(caps16) + FP8_COST * CF
        if best is None or cost < best[0]:
            best = (cost, CF, list(take), caps16, asg16)
    if best is None:
        return None
    _, CF, take, caps16, asg16 = best
    # slack-fill: spare p16 capacity absorbs the highest-weight fp8 visits
    for e in range(E):
        cb = sum(n * c for n, c in zip(asg16[e], caps16))
        slack = cb - (n_e[e] - take[e])
        if slack > 0:
            take[e] = max(0, take[e] - slack)
    return CF, take, caps16, asg16


# --------------------------------------------------------------------------
# device kernel
# --------------------------------------------------------------------------

def _groups(cap):
    gs = [512] * (cap // 512)
    if cap % 512:
        gs.append(cap % 512)
    offs = np.cumsum([0] + gs)[:-1]
    return list(zip(offs, gs))


def _build(slot_desc):
    """slot_desc: tuple of (cap, cls) in device order."""
    nc = bacc_mod.Bacc("TRN2")

    def dram(name, shape, dtype, out=False):
        return nc.declare_dram_parameter(name, list(shape), dtype, isOutput=out)

    slots = []
    for j, (cap, cls) in enumerate(slot_desc):
        p = f"s{j}"
        dt = CLS[cls]["dt"]
        w = {
            "xt": dram(p + "xt", [128, DK, cap], dt),
            "wf": dram(p + "wf", [IT, 128, 4 * DK, 128], dt),
            "w2": dram(p + "w2", [DK, 128, IT, 128], dt),
            "bias": dram(p + "bias", [128, NB], F32),
            "y": dram(p + "y", [DK, 128, cap], FP16, out=True),
        }
        slots.append((j, cap, cls, w))

    with tile.TileContext(nc) as tc:
        with (
            tc.tile_pool(name="persist", bufs=1) as persist,
            tc.tile_pool(name="wpool", bufs=6) as wpool,
            tc.tile_pool(name="w2pool", bufs=6) as w2pool,
            tc.tile_pool(name="work", bufs=2) as work,
            tc.tile_pool(name="outp", bufs=3) as outp,
            tc.tile_pool(name="ps", bufs=1, space="PSUM") as ps,
            tc.tile_pool(name="psy", bufs=3, space="PSUM") as psy,
        ):
            xts_t, bt_t, hb_t = {}, {}, {}
            for j, cap, cls, w in slots:
                dt = CLS[cls]["dt"]
                xts_t[j] = persist.tile([128, DK, cap], dt, tag=f"xt{j}",
                                        name=f"xt_s{j}")
                bt_t[j] = persist.tile([128, NB], F32, tag=f"bias{j}",
                                       name=f"bias_s{j}")
                hb_t[j] = persist.tile([128, IT, cap], dt, tag=f"h{j}",
                                       name=f"h_s{j}")

            def load_xt_bias(j):
                _, cap, _, w = slots[j]
                if j == 0:
                    half = DK // 2
                    xap = w["xt"].ap()
                    nc.gpsimd.dma_start(out=xts_t[j][:, :half],
                                        in_=xap[:, :half])
                    nc.gpsimd.dma_start(out=xts_t[j][:, half:],
                                        in_=xap[:, half:])
                else:
                    nc.gpsimd.dma_start(out=xts_t[j], in_=w["xt"].ap())
                nc.gpsimd.dma_start(out=bt_t[j], in_=w["bias"].ap())

            load_xt_bias(0)

            def wf_load(j, it):
                _, _, cls, w = slots[j]
                dt = CLS[cls]["dt"]
                wt = wpool.tile([128, 4 * DK, 128], dt, tag="wf",
                                name=f"wf_s{j}_{it}")
                if j == 0 and it <= 2:
                    for wi in range(4):
                        nc.sync.dma_start(
                            out=wt[:, wi * DK:(wi + 1) * DK, :],
                            in_=w["wf"][it][:, wi * DK:(wi + 1) * DK, :])
                else:
                    nc.sync.dma_start(out=wt, in_=w["wf"][it])
                return wt

            wf_pre = {}
            for j, cap, cls, w in slots:
                c = CLS[cls]
                s1 = c["SX"] * c["SW"]
                dbl = c["dbl"]
                grp = _groups(cap)
                xts, bt, hb = xts_t[j], bt_t[j], hb_t[j]
                w2_pre = {}

                def prefetch_w2(dk, j=j, cls=cls, w=w, w2_pre=w2_pre):
                    w2t = w2pool.tile([128, IT, 128], CLS[cls]["dt"], tag="w2",
                                      name=f"w2_s{j}_{dk}")
                    nc.sync.dma_start(out=w2t, in_=w["w2"][dk])
                    w2_pre[dk] = w2t

                # ---- first GEMM + swiglu: h[it, tok] ----
                for it in range(IT):
                    wt = wf_pre.pop((j, it), None)
                    if wt is None:
                        wt = wf_load(j, it)
                    if it == 8 and j + 1 < len(slots):
                        load_xt_bias(j + 1)
                    if it in (5, 7, 9, 11, 13, 15):
                        prefetch_w2((it - 5) // 2)
                    for goff, gsz in grp:
                        accs = []
                        for wi in range(4):
                            acc = ps.tile([128, 512], F32, tag=f"acc{wi}",
                                          name=f"acc{wi}_s{j}_{it}_{goff}")
                            if dbl:
                                for p2 in range(DK // 2):
                                    nc.tensor.matmul(
                                        acc[:, :gsz],
                                        wt[:, wi * DK + 2 * p2:
                                           wi * DK + 2 * p2 + 2, :],
                                        xts[:, 2 * p2:2 * p2 + 2,
                                            goff:goff + gsz],
                                        start=(p2 == 0),
                                        stop=(p2 == DK // 2 - 1),
                                        perf_mode=mybir.MatmulPerfMode.DoubleRow)
                            else:
                                for dk in range(DK):
                                    nc.tensor.matmul(
                                        acc[:, :gsz],
                                        wt[:, wi * DK + dk, :],
                                        xts[:, dk, goff:goff + gsz],
                                        start=(dk == 0), stop=(dk == DK - 1))
                            accs.append(acc)
                        A, Bm, C, Dm = accs
                        # Bp = v_e = B/s1 + b3e
                        Bp = work.tile([128, 512], F32, tag="Bp")
                        nc.scalar.activation(Bp[:, :gsz], Bm[:, :gsz],
                                             Act.Identity, scale=1.0 / s1,
                                             bias=bt[:, IT + it:IT + it + 1])
                        # G = (A + s1*b1e) * Bp = s1*g
                        G = work.tile([128, 512], F32, tag="G")
                        nc.vector.scalar_tensor_tensor(
                            G[:, :gsz], A[:, :gsz], bt[:, it:it + 1],
                            Bp[:, :gsz], Alu.add, Alu.mult)
                        nc.vector.tensor_scalar_min(G[:, :gsz], G[:, :gsz],
                                                    LIMIT * s1)
                        # Sv = Silu(alpha*g) = alpha*g*sig(alpha*g)
                        Sv = work.tile([128, 512], F32, tag="Sv")
                        nc.scalar.activation(Sv[:, :gsz], G[:, :gsz],
                                             Act.Silu, scale=ALPHA / s1)
                        # Dp = (SH/s1)*v_o
                        Dp = work.tile([128, 512], F32, tag="Dp")
                        nc.scalar.activation(
                            Dp[:, :gsz], Dm[:, :gsz], Act.Identity,
                            scale=c["SH"] / (s1 * s1),
                            bias=bt[:, 3 * IT + it:3 * IT + it + 1])
                        # L = (C + s1*b1o) * Dp = SH*l
                        L = work.tile([128, 512], F32, tag="L")
                        nc.vector.scalar_tensor_tensor(
                            L[:, :gsz], C[:, :gsz],
                            bt[:, 2 * IT + it:2 * IT + it + 1],
                            Dp[:, :gsz], Alu.add, Alu.mult)
                        nc.vector.tensor_scalar(L[:, :gsz], L[:, :gsz],
                                                LIMIT * c["SH"],
                                                -LIMIT * c["SH"],
                                                Alu.min, Alu.max)
                        # hb = (L + SH)*Sv = SH*alpha*h
                        nc.vector.scalar_tensor_tensor(
                            hb[:, it, goff:goff + gsz],
                            L[:, :gsz], c["SH"], Sv[:, :gsz],
                            Alu.add, Alu.mult)

                if j + 1 < len(slots):
                    for it2 in range(5):
                        wf_pre[(j + 1, it2)] = wf_load(j + 1, it2)

                # ---- second GEMM: y[dk] = sum_it w2[dk,it].T @ h[it] ----
                y_scale = 1.0 / (c["SH"] * c["SW2"])
                for dk in range(DK):
                    if dk in w2_pre:
                        w2t = w2_pre.pop(dk)
                    else:
                        w2t = w2pool.tile([128, IT, 128], c["dt"], tag="w2",
                                          name=f"w2_s{j}_{dk}")
                        nc.sync.dma_start(out=w2t, in_=w["w2"][dk])
                    for goff, gsz in grp:
                        Y = psy.tile([128, 512], F32, tag="Y",
                                     name=f"Y_s{j}_{dk}_{goff}")
                        if dbl:
                            for p2 in range(IT // 2):
                                nc.tensor.matmul(
                                    Y[:, :gsz],
                                    w2t[:, 2 * p2:2 * p2 + 2, :],
                                    hb[:, 2 * p2:2 * p2 + 2, goff:goff + gsz],
                                    start=(p2 == 0), stop=(p2 == IT // 2 - 1),
                                    perf_mode=mybir.MatmulPerfMode.DoubleRow)
                        else:
                            for it in range(IT):
                                nc.tensor.matmul(
                                    Y[:, :gsz],
                                    w2t[:, it, :],
                                    hb[:, it, goff:goff + gsz],
                                    start=(it == 0), stop=(it == IT - 1))
                        yo = outp.tile([128, 512], FP16, tag="yo")
                        nc.scalar.activation(
                            yo[:, :gsz], Y[:, :gsz], Act.Identity,
                            scale=y_scale,
                            bias=bt[:, 4 * IT + dk:4 * IT + dk + 1])
                        nc.scalar.dma_start(
                            out=w["y"][dk, :, goff:goff + gsz],
                            in_=yo[:, :gsz])

    nc.finalize()
    return nc


# --------------------------------------------------------------------------
# host plan construction (shared with the accuracy sim)
# --------------------------------------------------------------------------

def _make_plan(xt, gate_w, gate_b):
    """Gate + routing + slot planning.  Returns a dict with everything the
    packer/combiner needs."""
    z = xt.astype(np.float64) @ np.asarray(gate_w, dtype=np.float64).T
    z -= z.max(axis=-1, keepdims=True)
    ez = np.exp(z)
    scores = ez / ez.sum(axis=-1, keepdims=True)
    biased = scores + np.asarray(gate_b, dtype=np.float64)
    top2 = np.argsort(-biased, axis=-1, kind="stable")[:, :TOPK]
    gate_wt = np.take_along_axis(scores, top2, axis=-1).astype(np.float32)

    tok_sorted, wt_sorted = [], []
    for e in range(E):
        sel = np.nonzero((top2 == e).any(axis=1))[0]
        we = np.where(top2[sel, 0] == e, gate_wt[sel, 0],
                      gate_wt[sel, 1]).astype(np.float32)
        order = np.argsort(we, kind="stable")
        tok_sorted.append(sel[order])
        wt_sorted.append(we[order])
    n_e = [len(s) for s in tok_sorted]

    plan = _plan_hybrid(n_e, wt_sorted)
    if plan is None:
        raise RuntimeError("no hybrid plan found")
    CF, take8, caps16, asg16 = plan

    # p16 pieces per cap position
    pieces16 = {jj: [] for jj in range(len(caps16))}
    for e in range(E):
        lo = take8[e]
        for jj in range(len(caps16)):
            for _ in range(asg16[e][jj]):
                hi = min(lo + caps16[jj], n_e[e])
                pieces16[jj].append((e, lo, hi))
                lo = hi
        assert lo >= n_e[e]
    for jj in range(len(caps16)):
        while len(pieces16[jj]) < N_CORES:
            pieces16[jj].append((0, 0, 0))

    pieces8 = [(e, 0, take8[e]) for e in range(E)]

    # device slot order: by estimated compute, descending
    entries = [("sh", TS, "p16")]
    entries += [(("p16", jj), caps16[jj], "p16") for jj in range(len(caps16))]
    entries += [(("p8", 0), CF, "p8")]
    entries.sort(key=lambda t: -(t[1] * (FP8_COST if t[2] == "p8" else 1.0)))

    return dict(top2=top2, gate_wt=gate_wt, tok_sorted=tok_sorted,
                wt_sorted=wt_sorted, n_e=n_e, CF=CF, take8=take8,
                caps16=caps16, pieces16=pieces16, pieces8=pieces8,
                entries=entries)


# --------------------------------------------------------------------------
# entry point
# --------------------------------------------------------------------------

def kernel(x, gate_w, gate_b, w1, b1, w3, b3, w2, b2,
           sw1, sb1, sw3, sb3, sw2, sb2):
    x = np.asarray(x, dtype=np.float32)
    xt = x.reshape(T, D)

    plan = _make_plan(xt, gate_w, gate_b)
    entries = plan["entries"]
    pieces16, pieces8 = plan["pieces16"], plan["pieces8"]
    tok_sorted, wt_sorted = plan["tok_sorted"], plan["wt_sorted"]

    epacks = {}

    def epack(e, cls):
        if (e, cls) not in epacks:
            epacks[(e, cls)] = _expert_pack(
                np.asarray(w1[e]), np.asarray(b1[e]), np.asarray(w3[e]),
                np.asarray(b3[e]), np.asarray(w2[e]), np.asarray(b2[e]), cls)
        return epacks[(e, cls)]

    spack = _expert_pack(np.asarray(sw1), np.asarray(sb1), np.asarray(sw3),
                         np.asarray(sb3), np.asarray(sw2), np.asarray(sb2),
                         "p16")

    in_maps = []
    for cc in range(N_CORES):
        m = {}
        for s, (kind, cap, cls) in enumerate(entries):
            if kind == "sh":
                m[f"s{s}xt"] = _xt_pack(xt[cc * TS:(cc + 1) * TS], TS, cls)
                pk = spack
            elif kind[0] == "p16":
                e, lo, hi = pieces16[kind[1]][cc]
                m[f"s{s}xt"] = _xt_pack(xt[tok_sorted[e][lo:hi]], cap, cls)
                pk = epack(e, cls)
            else:
                e, lo, hi = pieces8[cc]
                m[f"s{s}xt"] = _xt_pack(xt[tok_sorted[e][lo:hi]], cap, cls)
                pk = epack(e, cls)
            for kk, v in pk.items():
                m[f"s{s}{kk}"] = v
        in_maps.append(m)

    slot_desc = tuple((cap, cls) for _, cap, cls in entries)
    if slot_desc not in _kernel_cache:
        _kernel_cache[slot_desc] = _build(slot_desc)
    nc = _kernel_cache[slot_desc]
    res = run_bass_kernel_spmd(nc, in_maps, list(range(N_CORES)))

    # ---- combine ----
    out = np.zeros((T, D), dtype=np.float32)
    for cc in range(N_CORES):
        for s, (kind, cap, cls) in enumerate(entries):
            yc = res.results[cc][f"s{s}y"].astype(np.float32)
            yc = yc.reshape(D, cap)
            if kind == "sh":
                out[cc * TS:(cc + 1) * TS] += yc.T
            else:
                if kind[0] == "p16":
                    e, lo, hi = pieces16[kind[1]][cc]
                else:
                    e, lo, hi = pieces8[cc]
                if hi <= lo:
                    continue
                idx = tok_sorted[e][lo:hi]
                out[idx] += wt_sorted[e][lo:hi][:, None] * yc.T[:hi - lo]
    return out.reshape(B, S, D)


# revision 8
# speedup vs baseline: 1.1833x; 1.0667x over previous
"""MoE (top-2 of 8 experts + shared expert) Trainium2 kernel, expert-parallel
across 8 NeuronCores, hybrid fp16/fp8 precision.

Strategy (upgrade over the 445us bf16 baseline):
  - Host: gate in float64 numpy; tokens dispatched by routing index.
  - Precision hybrid: each routed expert-visit's error contribution to the
    final output is attenuated by its gate combine weight (mean ~0.24), while
    the shared expert enters with weight 1.  So the lowest-weight ~2/3 of
    routed visits (chosen by a global sum-w^2 error budget FRAC) run fully in
    fp8 e4m3 with DoubleRow matmuls (2x PE rate, measured 1.92x), and the
    high-weight rest + the shared expert run in fp16 (same PE rate as bf16,
    4 more mantissa bits -> smaller base error).
  - Quantization scales (w x32, x x8 for fp8, h x2) keep values out of fp8/
    fp16 denormals and are folded into ACT scale/bias constants -- zero extra
    device ops vs the baseline swiglu (5 DVE + 3 ACT per i-tile).
  - Per-core slots: [shared 512 f16] + p16 routed slot(s) (caps from a DP
    cover of the residual per-expert counts) + one fp8 slot (cap CF, one
    expert per core, so its weights stream once).  Planner minimizes
    sum(caps16) + 0.52*CF subject to the error budget.
  - Scheduling: largest-compute slot first; x/bias loads on the GpSimd queue,
    y writebacks on the Scalar queue, weight loads on Sync; next-slot x at
    it==8, 5 wf tiles prefetched ahead of each slot's GEMM2, w2 prefetched
    at odd i-tiles.
"""
import sys

sys.path.insert(0, "/opt/trn_rl_repo")

import itertools
import os

import ml_dtypes
import numpy as np

import concourse.bacc as bacc_mod
import concourse.tile as tile
from concourse import mybir
from concourse.bass_utils import run_bass_kernel_spmd

F32 = mybir.dt.float32
FP16 = mybir.dt.float16
FP8 = mybir.dt.float8e4
NP_F8 = ml_dtypes.float8_e4m3
Alu = mybir.AluOpType
Act = mybir.ActivationFunctionType

ALPHA = 1.702
LIMIT = 7.0
TOPK = 2
D, I, E = 1024, 2048, 8
B, S = 2, 2048
T = B * S
DK = D // 128          # 8 d-tiles
IT = I // 128          # 16 i-tiles
TS = 512               # shared-expert tokens per core (T / 8)
N_CORES = 8
NB = 4 * IT + DK       # bias-pack columns

# fraction of routed sum-w^2 allowed into fp8 (error budget)
FRAC = float(os.environ.get("MOE_FRAC", "0.50"))
FP8_COST = 0.52        # measured fp8 PE cost per token vs fp16
DMA16 = 218            # p16 slot weight-stream floor, token units (21MB)
DMA8 = 109             # fp8 slot weight-stream floor (10.5MB)
SLOT_OH = 20           # per-slot pipeline overhead, token units

# per-class constants: s1 = SX*SW is the GEMM1 psum scale
CLS = {
    "p16": dict(dt=FP16, npdt=np.float16, SX=1.0, SW=32.0, SH=1.0, SW2=32.0,
                dbl=False),
    "p8": dict(dt=FP8, npdt=NP_F8, SX=8.0, SW=32.0, SH=2.0, SW2=32.0,
               dbl=True),
}

_kernel_cache = {}


# --------------------------------------------------------------------------
# host-side packing
# --------------------------------------------------------------------------

def _q(a, cls):
    if cls == "p8":
        return np.clip(a, -240.0, 240.0).astype(NP_F8)
    return a.astype(np.float16)


def _tile13(w):
    """[D, I] -> [IT, 128(k), DK, 128(m)]."""
    return w.reshape(DK, 128, IT, 128).transpose(2, 1, 0, 3)


def _expert_pack(w1, b1, w3, b3, w2, b2, cls):
    c = CLS[cls]
    s1 = c["SX"] * c["SW"]
    wf = np.stack([_tile13(w1[:, 0::2]), _tile13(w3[:, 0::2]),
                   _tile13(w1[:, 1::2]), _tile13(w3[:, 1::2])], axis=2)
    wf = np.ascontiguousarray(wf.reshape(IT, 128, 4 * DK, 128)) * c["SW"]
    w2t = (w2 * (c["SW2"] / ALPHA)).reshape(IT, 128, DK, 128)
    w2t = np.ascontiguousarray(w2t.transpose(2, 1, 0, 3))  # [DK,128,IT,128]
    bias = np.concatenate([
        s1 * b1[0::2].reshape(IT, 128).T,
        b3[0::2].reshape(IT, 128).T,
        s1 * b1[1::2].reshape(IT, 128).T,
        (c["SH"] / s1) * b3[1::2].reshape(IT, 128).T,
        b2.reshape(DK, 128).T,
    ], axis=1)
    return {
        "wf": _q(wf, cls),
        "w2": _q(w2t, cls),
        "bias": np.ascontiguousarray(bias, dtype=np.float32),
    }


def _xt_pack(xsub, cap, cls):
    """[n, D] tokens -> zero-padded [128, DK, cap] transposed layout."""
    c = CLS[cls]
    n = xsub.shape[0]
    xt = np.zeros((D, cap), dtype=np.float32)
    xt[:, :n] = (c["SX"] * xsub).T
    xt = np.ascontiguousarray(xt.reshape(DK, 128, cap).transpose(1, 0, 2))
    return _q(xt, cls)


# --------------------------------------------------------------------------
# planning
# --------------------------------------------------------------------------

def _cover(caps, counts, ninst=8):
    """Assign instances of each cap to experts covering counts."""
    k = len(caps)
    per = []
    for cnt in counts:
        out = []
        for combo in itertools.product(range(ninst + 1), repeat=k):
            cap = sum(n * c for n, c in zip(combo, caps))
            if cap >= cnt:
                out.append((cap - cnt, combo))
        if not out:
            return None
        out.sort()
        per.append([c for _, c in out[:64]])
    memo = {}

    def dp(i, used):
        if i == len(counts):
            return []
        key = (i, used)
        if key in memo:
            return memo[key]
        res = None
        for combo in per[i]:
            nu = tuple(u + n for u, n in zip(used, combo))
            if any(u > ninst for u in nu):
                continue
            sub = dp(i + 1, nu)
            if sub is not None:
                res = [combo] + sub
                break
        memo[key] = res
        return res

    return dp(0, (0,) * k)


def _plan_hybrid(n_e, wt_sorted):
    """Cost-ordered search over slot structures (cf | c1 | c2) with a DP
    budget-feasibility check.  Slot cost = max(PE, DMA floor) + overhead.
    Returns (cf, caps16, per-expert (i8, i1, i2))."""
    cum = [np.concatenate([[0.0], np.cumsum(w.astype(np.float64) ** 2)])
           for w in wt_sorted]
    budget = FRAC * sum(float(c[-1]) for c in cum)
    tot = sum(n_e)

    cands = []
    for cf in range(0, 1025, 32):
        cost8 = (max(FP8_COST * cf, DMA8) + SLOT_OH) if cf else 0.0
        for c1 in range(224, 513, 16):
            for c2 in [0] + list(range(224, c1 + 1, 16)):
                cost = cost8 + max(c1, DMA16) + SLOT_OH
                if c2:
                    cost += max(c2, DMA16) + SLOT_OH
                cands.append((cost, -cf, c1, c2))
    cands.sort()
    for cost, ncf, c1, c2 in cands:
        cf = -ncf
        if 8 * (cf + c1 + c2) < tot:
            continue
        asg = _assign_chain(cf, c1, c2, n_e, cum, budget)
        if asg is not None:
            return cf, [c1] + ([c2] if c2 else []), asg
    return None


def _assign_chain(cf, c1, c2, n_e, cum, budget):
    """DP with full chain tracking (small state count)."""
    states = {(0, 0, 0): (0.0, [])}
    order = sorted(range(E), key=lambda e: -n_e[e])
    for e in order:
        ne = n_e[e]
        combos = []
        seen = set()
        for i1 in range(9):
            for i2 in range(9 if c2 else 1):
                pc = i1 * c1 + i2 * c2
                f8 = max(0, ne - pc)
                if cf:
                    i8 = -(-f8 // cf)
                elif f8:
                    continue
                else:
                    i8 = 0
                if i8 > 8:
                    continue
                key = (i8, i1, i2)
                if key in seen:
                    continue
                seen.add(key)
                combos.append((i8, i1, i2, float(cum[e][f8])))
        if not combos:
            return None
        nxt = {}
        for (u8, u1, u2), (val, chain) in states.items():
            for i8, i1, i2, w2v in combos:
                n8, n1, n2 = u8 + i8, u1 + i1, u2 + i2
                if n8 > 8 or n1 > 8 or n2 > 8:
                    continue
                nv = val + w2v
                if nv > budget:
                    continue
                k = (n8, n1, n2)
                if k not in nxt or nv < nxt[k][0]:
                    nxt[k] = (nv, chain + [(e, i8, i1, i2)])
        if not nxt:
            return None
        states = nxt
    best = min(states.values(), key=lambda v: v[0])
    out = [None] * E
    for e, i8, i1, i2 in best[1]:
        out[e] = (i8, i1, i2)
    return out


# --------------------------------------------------------------------------
# device kernel
# --------------------------------------------------------------------------

def _groups(cap):
    gs = [512] * (cap // 512)
    if cap % 512:
        gs.append(cap % 512)
    offs = np.cumsum([0] + gs)[:-1]
    return list(zip(offs, gs))


def _build(slot_desc):
    """slot_desc: tuple of (cap, cls) in device order."""
    nc = bacc_mod.Bacc("TRN2")

    def dram(name, shape, dtype, out=False):
        return nc.declare_dram_parameter(name, list(shape), dtype, isOutput=out)

    slots = []
    for j, (cap, cls) in enumerate(slot_desc):
        p = f"s{j}"
        dt = CLS[cls]["dt"]
        w = {
            "xt": dram(p + "xt", [128, DK, cap], dt),
            "wf": dram(p + "wf", [IT, 128, 4 * DK, 128], dt),
            "w2": dram(p + "w2", [DK, 128, IT, 128], dt),
            "bias": dram(p + "bias", [128, NB], F32),
            "y": dram(p + "y", [DK, 128, cap], FP16, out=True),
        }
        slots.append((j, cap, cls, w))

    with tile.TileContext(nc) as tc:
        with (
            tc.tile_pool(name="persist", bufs=1) as persist,
            tc.tile_pool(name="wpool", bufs=6) as wpool,
            tc.tile_pool(name="w2pool", bufs=8) as w2pool,
            tc.tile_pool(name="work", bufs=2) as work,
            tc.tile_pool(name="outp", bufs=3) as outp,
            tc.tile_pool(name="ps", bufs=1, space="PSUM") as ps,
            tc.tile_pool(name="psy", bufs=3, space="PSUM") as psy,
        ):
            xts_t, bt_t, hb_t = {}, {}, {}
            for j, cap, cls, w in slots:
                dt = CLS[cls]["dt"]
                xts_t[j] = persist.tile([128, DK, cap], dt, tag=f"xt{j}",
                                        name=f"xt_s{j}")
                bt_t[j] = persist.tile([128, NB], F32, tag=f"bias{j}",
                                       name=f"bias_s{j}")
                hb_t[j] = persist.tile([128, IT, cap], dt, tag=f"h{j}",
                                       name=f"h_s{j}")

            def load_xt_bias(j):
                _, cap, _, w = slots[j]
                if j == 0:
                    half = DK // 2
                    xap = w["xt"].ap()
                    nc.gpsimd.dma_start(out=xts_t[j][:, :half],
                                        in_=xap[:, :half])
                    nc.gpsimd.dma_start(out=xts_t[j][:, half:],
                                        in_=xap[:, half:])
                else:
                    nc.gpsimd.dma_start(out=xts_t[j], in_=w["xt"].ap())
                nc.gpsimd.dma_start(out=bt_t[j], in_=w["bias"].ap())

            load_xt_bias(0)

            def wf_load(j, it):
                _, _, cls, w = slots[j]
                dt = CLS[cls]["dt"]
                wt = wpool.tile([128, 4 * DK, 128], dt, tag="wf",
                                name=f"wf_s{j}_{it}")
                if j == 0 and it <= 2:
                    for wi in range(4):
                        nc.sync.dma_start(
                            out=wt[:, wi * DK:(wi + 1) * DK, :],
                            in_=w["wf"][it][:, wi * DK:(wi + 1) * DK, :])
                else:
                    nc.sync.dma_start(out=wt, in_=w["wf"][it])
                return wt

            wf_pre = {}
            for j, cap, cls, w in slots:
                c = CLS[cls]
                s1 = c["SX"] * c["SW"]
                dbl = c["dbl"]
                grp = _groups(cap)
                xts, bt, hb = xts_t[j], bt_t[j], hb_t[j]
                w2_pre = {}

                def prefetch_w2(dk, j=j, cls=cls, w=w, w2_pre=w2_pre):
                    w2t = w2pool.tile([128, IT, 128], CLS[cls]["dt"], tag="w2",
                                      name=f"w2_s{j}_{dk}")
                    nc.sync.dma_start(out=w2t, in_=w["w2"][dk])
                    w2_pre[dk] = w2t

                # ---- first GEMM + swiglu: h[it, tok] ----
                for it in range(IT):
                    wt = wf_pre.pop((j, it), None)
                    if wt is None:
                        wt = wf_load(j, it)
                    if it == 8 and j + 1 < len(slots):
                        load_xt_bias(j + 1)
                    # all 8 w2 tiles land during GEMM1 so GEMM2 issues no
                    # Sync-queue loads behind the next slot's wf prefetch
                    W2_AT = {5: 0, 6: 1, 7: 2, 8: 3, 9: 4, 11: 5, 13: 6, 15: 7}
                    if it in W2_AT:
                        prefetch_w2(W2_AT[it])
                    for goff, gsz in grp:
                        accs = []
                        for wi in range(4):
                            acc = ps.tile([128, 512], F32, tag=f"acc{wi}",
                                          name=f"acc{wi}_s{j}_{it}_{goff}")
                            if dbl:
                                for p2 in range(DK // 2):
                                    nc.tensor.matmul(
                                        acc[:, :gsz],
                                        wt[:, wi * DK + 2 * p2:
                                           wi * DK + 2 * p2 + 2, :],
                                        xts[:, 2 * p2:2 * p2 + 2,
                                            goff:goff + gsz],
                                        start=(p2 == 0),
                                        stop=(p2 == DK // 2 - 1),
                                        perf_mode=mybir.MatmulPerfMode.DoubleRow)
                            else:
                                for dk in range(DK):
                                    nc.tensor.matmul(
                                        acc[:, :gsz],
                                        wt[:, wi * DK + dk, :],
                                        xts[:, dk, goff:goff + gsz],
                                        start=(dk == 0), stop=(dk == DK - 1))
                            accs.append(acc)
                        A, Bm, C, Dm = accs
                        # Bp = v_e = B/s1 + b3e
                        Bp = work.tile([128, 512], F32, tag="Bp")
                        nc.scalar.activation(Bp[:, :gsz], Bm[:, :gsz],
                                             Act.Identity, scale=1.0 / s1,
                                             bias=bt[:, IT + it:IT + it + 1])
                        # G = (A + s1*b1e) * Bp = s1*g
                        G = work.tile([128, 512], F32, tag="G")
                        nc.vector.scalar_tensor_tensor(
                            G[:, :gsz], A[:, :gsz], bt[:, it:it + 1],
                            Bp[:, :gsz], Alu.add, Alu.mult)
                        nc.vector.tensor_scalar_min(G[:, :gsz], G[:, :gsz],
                                                    LIMIT * s1)
                        # Sv = Silu(alpha*g) = alpha*g*sig(alpha*g)
                        Sv = work.tile([128, 512], F32, tag="Sv")
                        nc.scalar.activation(Sv[:, :gsz], G[:, :gsz],
                                             Act.Silu, scale=ALPHA / s1)
                        # Dp = (SH/s1)*v_o
                        Dp = work.tile([128, 512], F32, tag="Dp")
                        nc.scalar.activation(
                            Dp[:, :gsz], Dm[:, :gsz], Act.Identity,
                            scale=c["SH"] / (s1 * s1),
                            bias=bt[:, 3 * IT + it:3 * IT + it + 1])
                        # L = (C + s1*b1o) * Dp = SH*l
                        L = work.tile([128, 512], F32, tag="L")
                        nc.vector.scalar_tensor_tensor(
                            L[:, :gsz], C[:, :gsz],
                            bt[:, 2 * IT + it:2 * IT + it + 1],
                            Dp[:, :gsz], Alu.add, Alu.mult)
                        nc.vector.tensor_scalar(L[:, :gsz], L[:, :gsz],
                                                LIMIT * c["SH"],
                                                -LIMIT * c["SH"],
                                                Alu.min, Alu.max)
                        # hb = (L + SH)*Sv = SH*alpha*h
                        nc.vector.scalar_tensor_tensor(
                            hb[:, it, goff:goff + gsz],
                            L[:, :gsz], c["SH"], Sv[:, :gsz],
                            Alu.add, Alu.mult)

                if j + 1 < len(slots):
                    for it2 in range(5):
                        wf_pre[(j + 1, it2)] = wf_load(j + 1, it2)

                # ---- second GEMM: y[dk] = sum_it w2[dk,it].T @ h[it] ----
                y_scale = 1.0 / (c["SH"] * c["SW2"])
                for dk in range(DK):
                    if dk in w2_pre:
                        w2t = w2_pre.pop(dk)
                    else:
                        w2t = w2pool.tile([128, IT, 128], c["dt"], tag="w2",
                                          name=f"w2_s{j}_{dk}")
                        nc.sync.dma_start(out=w2t, in_=w["w2"][dk])
                    for goff, gsz in grp:
                        Y = psy.tile([128, 512], F32, tag="Y",
                                     name=f"Y_s{j}_{dk}_{goff}")
                        if dbl:
                            for p2 in range(IT // 2):
                                nc.tensor.matmul(
                                    Y[:, :gsz],
                                    w2t[:, 2 * p2:2 * p2 + 2, :],
                                    hb[:, 2 * p2:2 * p2 + 2, goff:goff + gsz],
                                    start=(p2 == 0), stop=(p2 == IT // 2 - 1),
                                    perf_mode=mybir.MatmulPerfMode.DoubleRow)
                        else:
                            for it in range(IT):
                                nc.tensor.matmul(
                                    Y[:, :gsz],
                                    w2t[:, it, :],
                                    hb[:, it, goff:goff + gsz],
                                    start=(it == 0), stop=(it == IT - 1))
                        yo = outp.tile([128, 512], FP16, tag="yo")
                        nc.scalar.activation(
                            yo[:, :gsz], Y[:, :gsz], Act.Identity,
                            scale=y_scale,
                            bias=bt[:, 4 * IT + dk:4 * IT + dk + 1])
                        nc.scalar.dma_start(
                            out=w["y"][dk, :, goff:goff + gsz],
                            in_=yo[:, :gsz])

    nc.finalize()
    return nc


# --------------------------------------------------------------------------
# host plan construction (shared with the accuracy sim)
# --------------------------------------------------------------------------

def _make_plan(xt, gate_w, gate_b):
    """Gate + routing + slot planning.  Returns a dict with everything the
    packer/combiner needs."""
    z = xt.astype(np.float64) @ np.asarray(gate_w, dtype=np.float64).T
    z -= z.max(axis=-1, keepdims=True)
    ez = np.exp(z)
    scores = ez / ez.sum(axis=-1, keepdims=True)
    biased = scores + np.asarray(gate_b, dtype=np.float64)
    top2 = np.argsort(-biased, axis=-1, kind="stable")[:, :TOPK]
    gate_wt = np.take_along_axis(scores, top2, axis=-1).astype(np.float32)

    tok_sorted, wt_sorted = [], []
    for e in range(E):
        sel = np.nonzero((top2 == e).any(axis=1))[0]
        we = np.where(top2[sel, 0] == e, gate_wt[sel, 0],
                      gate_wt[sel, 1]).astype(np.float32)
        order = np.argsort(we, kind="stable")
        tok_sorted.append(sel[order])
        wt_sorted.append(we[order])
    n_e = [len(s) for s in tok_sorted]

    plan = _plan_hybrid(n_e, wt_sorted)
    if plan is None:
        raise RuntimeError("no hybrid plan found")
    CF, caps16, asg = plan

    # token split per expert: bottom take8 go fp8, rest p16 (p16-first fill)
    take8 = []
    for e in range(E):
        i8, i1, i2 = asg[e]
        pc = i1 * caps16[0] + (i2 * caps16[1] if len(caps16) > 1 else 0)
        take8.append(max(0, n_e[e] - pc))

    # p16 pieces per cap position (larger cap filled first)
    pieces16 = {jj: [] for jj in range(len(caps16))}
    for e in range(E):
        i8, i1, i2 = asg[e]
        lo = take8[e]
        for jj, ni in enumerate([i1, i2][:len(caps16)]):
            for _ in range(ni):
                hi = min(lo + caps16[jj], n_e[e])
                pieces16[jj].append((e, lo, hi))
                lo = hi
        assert lo >= n_e[e]
    for jj in range(len(caps16)):
        while len(pieces16[jj]) < N_CORES:
            pieces16[jj].append((0, 0, 0))

    # fp8 pieces (an expert may own several instances)
    pieces8 = []
    for e in range(E):
        i8 = asg[e][0]
        lo = 0
        for _ in range(i8):
            hi = min(lo + CF, take8[e])
            pieces8.append((e, lo, hi))
            lo = hi
        assert lo >= take8[e]
    while len(pieces8) < N_CORES:
        pieces8.append((0, 0, 0))

    # device slot order: fp8 first (cheap fill), p16 middle, shared last
    entries = []
    if CF:
        entries.append((("p8", 0), CF, "p8"))
    entries += [(("p16", jj), caps16[jj], "p16") for jj in range(len(caps16))]
    entries.append(("sh", TS, "p16"))

    return dict(top2=top2, gate_wt=gate_wt, tok_sorted=tok_sorted,
                wt_sorted=wt_sorted, n_e=n_e, CF=CF, take8=take8,
                caps16=caps16, pieces16=pieces16, pieces8=pieces8,
                entries=entries)


# --------------------------------------------------------------------------
# entry point
# --------------------------------------------------------------------------

def kernel(x, gate_w, gate_b, w1, b1, w3, b3, w2, b2,
           sw1, sb1, sw3, sb3, sw2, sb2):
    x = np.asarray(x, dtype=np.float32)
    xt = x.reshape(T, D)

    plan = _make_plan(xt, gate_w, gate_b)
    entries = plan["entries"]
    pieces16, pieces8 = plan["pieces16"], plan["pieces8"]
    tok_sorted, wt_sorted = plan["tok_sorted"], plan["wt_sorted"]

    epacks = {}

    def epack(e, cls):
        if (e, cls) not in epacks:
            epacks[(e, cls)] = _expert_pack(
                np.asarray(w1[e]), np.asarray(b1[e]), np.asarray(w3[e]),
                np.asarray(b3[e]), np.asarray(w2[e]), np.asarray(b2[e]), cls)
        return epacks[(e, cls)]

    spack = _expert_pack(np.asarray(sw1), np.asarray(sb1), np.asarray(sw3),
                         np.asarray(sb3), np.asarray(sw2), np.asarray(sb2),
                         "p16")

    in_maps = []
    for cc in range(N_CORES):
        m = {}
        for s, (kind, cap, cls) in enumerate(entries):
            if kind == "sh":
                m[f"s{s}xt"] = _xt_pack(xt[cc * TS:(cc + 1) * TS], TS, cls)
                pk = spack
            elif kind[0] == "p16":
                e, lo, hi = pieces16[kind[1]][cc]
                m[f"s{s}xt"] = _xt_pack(xt[tok_sorted[e][lo:hi]], cap, cls)
                pk = epack(e, cls)
            else:
                e, lo, hi = pieces8[cc]
                m[f"s{s}xt"] = _xt_pack(xt[tok_sorted[e][lo:hi]], cap, cls)
                pk = epack(e, cls)
            for kk, v in pk.items():
                m[f"s{s}{kk}"] = v
        in_maps.append(m)

    slot_desc = tuple((cap, cls) for _, cap, cls in entries)
    if slot_desc not in _kernel_cache:
        _kernel_cache[slot_desc] = _build(slot_desc)
    nc = _kernel_cache[slot_desc]
    res = run_bass_kernel_spmd(nc, in_maps, list(range(N_CORES)))

    # ---- combine ----
    out = np.zeros((T, D), dtype=np.float32)
    for cc in range(N_CORES):
        for s, (kind, cap, cls) in enumerate(entries):
            yc = res.results[cc][f"s{s}y"].astype(np.float32)
            yc = yc.reshape(D, cap)
            if kind == "sh":
                out[cc * TS:(cc + 1) * TS] += yc.T
            else:
                if kind[0] == "p16":
                    e, lo, hi = pieces16[kind[1]][cc]
                else:
                    e, lo, hi = pieces8[cc]
                if hi <= lo:
                    continue
                idx = tok_sorted[e][lo:hi]
                out[idx] += wt_sorted[e][lo:hi][:, None] * yc.T[:hi - lo]
    return out.reshape(B, S, D)


# revision 14
# speedup vs baseline: 1.2141x; 1.0261x over previous
"""MoE (top-2 of 8 experts + shared expert) Trainium2 kernel, expert-parallel
across 8 NeuronCores, hybrid fp16/fp8 precision.

Strategy (upgrade over the 445us bf16 baseline):
  - Host: gate in float64 numpy; tokens dispatched by routing index.
  - Precision hybrid: each routed expert-visit's error contribution to the
    final output is attenuated by its gate combine weight (mean ~0.24), while
    the shared expert enters with weight 1.  So the lowest-weight ~2/3 of
    routed visits (chosen by a global sum-w^2 error budget FRAC) run fully in
    fp8 e4m3 with DoubleRow matmuls (2x PE rate, measured 1.92x), and the
    high-weight rest + the shared expert run in fp16 (same PE rate as bf16,
    4 more mantissa bits -> smaller base error).
  - Quantization scales (w x32, x x8 for fp8, h x2) keep values out of fp8/
    fp16 denormals and are folded into ACT scale/bias constants -- zero extra
    device ops vs the baseline swiglu (5 DVE + 3 ACT per i-tile).
  - Per-core slots: [shared 512 f16] + p16 routed slot(s) (caps from a DP
    cover of the residual per-expert counts) + one fp8 slot (cap CF, one
    expert per core, so its weights stream once).  Planner minimizes
    sum(caps16) + 0.52*CF subject to the error budget.
  - Scheduling: largest-compute slot first; x/bias loads on the GpSimd queue,
    y writebacks on the Scalar queue, weight loads on Sync; next-slot x at
    it==8, 5 wf tiles prefetched ahead of each slot's GEMM2, w2 prefetched
    at odd i-tiles.
"""
import sys

sys.path.insert(0, "/opt/trn_rl_repo")

import itertools
import os

import ml_dtypes
import numpy as np

import concourse.bacc as bacc_mod
import concourse.tile as tile
from concourse import mybir
from concourse.bass_utils import run_bass_kernel_spmd

F32 = mybir.dt.float32
FP16 = mybir.dt.float16
FP8 = mybir.dt.float8e4
NP_F8 = ml_dtypes.float8_e4m3
Alu = mybir.AluOpType
Act = mybir.ActivationFunctionType

ALPHA = 1.702
LIMIT = 7.0
TOPK = 2
D, I, E = 1024, 2048, 8
B, S = 2, 2048
T = B * S
DK = D // 128          # 8 d-tiles
IT = I // 128          # 16 i-tiles
TS = 512               # shared-expert tokens per core (T / 8)
N_CORES = 8
NB = 4 * IT + DK       # bias-pack columns

# fraction of routed sum-w^2 allowed into fp8 (error budget)
FRAC = float(os.environ.get("MOE_FRAC", "0.50"))
FP8_COST = 0.52        # measured fp8 PE cost per token vs fp16
DMA16 = 218            # p16 slot weight-stream floor, token units (21MB)
DMA8 = 109             # fp8 slot weight-stream floor (10.5MB)
SLOT_OH = 16           # per-slot pipeline overhead, token units

# per-class constants: s1 = SX*SW is the GEMM1 psum scale
CLS = {
    "p16": dict(dt=FP16, npdt=np.float16, SX=1.0, SW=32.0, SH=1.0, SW2=32.0,
                dbl=False),
    "p8": dict(dt=FP8, npdt=NP_F8, SX=8.0, SW=32.0, SH=2.0, SW2=32.0,
               dbl=True),
}

_kernel_cache = {}


# --------------------------------------------------------------------------
# host-side packing
# --------------------------------------------------------------------------

def _q(a, cls):
    if cls == "p8":
        return np.clip(a, -240.0, 240.0).astype(NP_F8)
    return a.astype(np.float16)


def _tile13(w):
    """[D, I] -> [IT, 128(k), DK, 128(m)]."""
    return w.reshape(DK, 128, IT, 128).transpose(2, 1, 0, 3)


def _expert_pack(w1, b1, w3, b3, w2, b2, cls):
    c = CLS[cls]
    s1 = c["SX"] * c["SW"]
    wf = np.stack([_tile13(w1[:, 0::2]), _tile13(w3[:, 0::2]),
                   _tile13(w1[:, 1::2]), _tile13(w3[:, 1::2])], axis=2)
    wf = np.ascontiguousarray(wf.reshape(IT, 128, 4 * DK, 128)) * c["SW"]
    w2t = (w2 * (c["SW2"] / ALPHA)).reshape(IT, 128, DK, 128)
    w2t = np.ascontiguousarray(w2t.transpose(2, 1, 0, 3))  # [DK,128,IT,128]
    bias = np.concatenate([
        s1 * b1[0::2].reshape(IT, 128).T,
        b3[0::2].reshape(IT, 128).T,
        s1 * b1[1::2].reshape(IT, 128).T,
        (c["SH"] / s1) * b3[1::2].reshape(IT, 128).T,
        b2.reshape(DK, 128).T,
    ], axis=1)
    return {
        "wf": _q(wf, cls),
        "w2": _q(w2t, cls),
        "bias": np.ascontiguousarray(bias, dtype=np.float32),
    }


def _xt_pack(xsub, cap, cls):
    """[n, D] tokens -> zero-padded [128, DK, cap] transposed layout."""
    c = CLS[cls]
    n = xsub.shape[0]
    xt = np.zeros((D, cap), dtype=np.float32)
    xt[:, :n] = (c["SX"] * xsub).T
    xt = np.ascontiguousarray(xt.reshape(DK, 128, cap).transpose(1, 0, 2))
    return _q(xt, cls)


# --------------------------------------------------------------------------
# planning
# --------------------------------------------------------------------------

def _cover(caps, counts, ninst=8):
    """Assign instances of each cap to experts covering counts."""
    k = len(caps)
    per = []
    for cnt in counts:
        out = []
        for combo in itertools.product(range(ninst + 1), repeat=k):
            cap = sum(n * c for n, c in zip(combo, caps))
            if cap >= cnt:
                out.append((cap - cnt, combo))
        if not out:
            return None
        out.sort()
        per.append([c for _, c in out[:64]])
    memo = {}

    def dp(i, used):
        if i == len(counts):
            return []
        key = (i, used)
        if key in memo:
            return memo[key]
        res = None
        for combo in per[i]:
            nu = tuple(u + n for u, n in zip(used, combo))
            if any(u > ninst for u in nu):
                continue
            sub = dp(i + 1, nu)
            if sub is not None:
                res = [combo] + sub
                break
        memo[key] = res
        return res

    return dp(0, (0,) * k)


def _slot_cost(cap, cls):
    if cls == "p8":
        return max(FP8_COST * cap, DMA8) + SLOT_OH
    return max(cap, DMA16) + SLOT_OH


def _expert_combos(positions, ne, cume):
    """Instance-count combos (one tuple per position) covering ne tokens,
    p16-first fill; value = fp8 sum-w^2."""
    i16 = [i for i, (c, k) in enumerate(positions) if k == "p16"]
    i8s = [i for i, (c, k) in enumerate(positions) if k == "p8"]
    out = []
    seen = set()
    ranges = [range(9)] * len(i16)
    for c16 in itertools.product(*ranges):
        pcap = sum(n * positions[i][0] for n, i in zip(c16, i16))
        f8 = max(0, ne - pcap)
        covers = []
        if f8 == 0:
            covers.append([0] * len(i8s))
        elif len(i8s) == 1:
            j = -(-f8 // positions[i8s[0]][0])
            if j <= 8:
                covers.append([j])
        elif len(i8s) == 2:
            ca, cb = positions[i8s[0]][0], positions[i8s[1]][0]
            for j1 in range(9):
                j2 = -(-max(0, f8 - j1 * ca) // cb)
                if j2 <= 8:
                    covers.append([j1, j2])
                    if j2 == 0:
                        break
        for cov in covers:
            ix = [0] * len(positions)
            for n, i in zip(c16, i16):
                ix[i] = n
            for n, i in zip(cov, i8s):
                ix[i] = n
            ixt = tuple(ix)
            if ixt in seen:
                continue
            seen.add(ixt)
            out.append((ixt, float(cume[f8])))
    return out


def _assign_np(positions, n_e, cum, budget):
    """Vectorized DP over instance-usage states; returns per-expert combo
    tuples or None."""
    P = len(positions)
    shape = (9,) * P
    INF = np.inf
    val = np.full(shape, INF)
    val[(0,) * P] = 0.0
    order = sorted(range(E), key=lambda e: -n_e[e])
    trace = []
    for e in order:
        combos = _expert_combos(positions, n_e[e], cum[e])
        if not combos:
            return None
        nv = np.full(shape, INF)
        pidx = np.full(shape, -1, dtype=np.int32)
        for ci, (ix, w2v) in enumerate(combos):
            src = val[tuple(slice(0, 9 - i) for i in ix)] + w2v
            dst = nv[tuple(slice(i, 9) for i in ix)]
            pv = pidx[tuple(slice(i, 9) for i in ix)]
            m = src < dst
            dst[m] = src[m]
            pv[m] = ci
        nv[nv > budget] = INF
        if not np.isfinite(nv).any():
            return None
        trace.append((e, combos, pidx))
        val = nv
    state = np.unravel_index(np.argmin(val), shape)
    if not np.isfinite(val[state]):
        return None
    out = [None] * E
    for e, combos, pidx in reversed(trace):
        ci = int(pidx[state])
        ix = combos[ci][0]
        out[e] = ix
        state = tuple(s - i for s, i in zip(state, ix))
    return out


def _plan_hybrid(n_e, wt_sorted):
    """Cost-ordered search over slot structures with DP budget feasibility.
    Returns (positions, asg): positions = [(cap, cls)...], asg[e] = instance
    counts per position."""
    cum = [np.concatenate([[0.0], np.cumsum(w.astype(np.float64) ** 2)])
           for w in wt_sorted]
    budget = FRAC * sum(float(c[-1]) for c in cum)
    tot = sum(n_e)

    f_grid = list(range(320, 897, 32))
    f2_grid = [0] + list(range(192, 897, 32))
    c_grid = list(range(224, 513, 16))
    cands = {}

    def add(poss):
        poss = tuple(sorted((p for p in poss if p[0] > 0),
                            key=lambda p: (p[1], -p[0])))
        if not poss or poss in cands:
            return
        if 8 * sum(c for c, _ in poss) < tot:
            return
        cands[poss] = sum(_slot_cost(c, k) for c, k in poss)

    for f1 in f_grid:
        for c1 in c_grid:
            add([(f1, "p8"), (c1, "p16")])
            for f2 in f2_grid:
                if f2 <= f1:
                    add([(f1, "p8"), (f2, "p8"), (c1, "p16")])
            for c2 in range(224, c1 + 1, 16):
                add([(f1, "p8"), (c1, "p16"), (c2, "p16")])
    for c1 in range(224, 513, 32):
        for c2 in range(224, c1 + 1, 32):
            add([(c1, "p16"), (c2, "p16")])
            for c3 in range(224, c2 + 1, 32):
                add([(c1, "p16"), (c2, "p16"), (c3, "p16")])

    for poss in sorted(cands, key=lambda p: cands[p]):
        asg = _assign_np(list(poss), n_e, cum, budget)
        if asg is not None:
            return list(poss), asg
    return None


# --------------------------------------------------------------------------
# device kernel
# --------------------------------------------------------------------------

def _groups(cap):
    gs = [512] * (cap // 512)
    if cap % 512:
        gs.append(cap % 512)
    offs = np.cumsum([0] + gs)[:-1]
    return list(zip(offs, gs))


def _build(slot_desc):
    """slot_desc: tuple of (cap, cls) in device order."""
    nc = bacc_mod.Bacc("TRN2")

    def dram(name, shape, dtype, out=False):
        return nc.declare_dram_parameter(name, list(shape), dtype, isOutput=out)

    slots = []
    for j, (cap, cls) in enumerate(slot_desc):
        p = f"s{j}"
        dt = CLS[cls]["dt"]
        w = {
            "xt": dram(p + "xt", [128, DK, cap], dt),
            "wf": dram(p + "wf", [IT, 128, 4 * DK, 128], dt),
            "w2": dram(p + "w2", [DK, 128, IT, 128], dt),
            "bias": dram(p + "bias", [128, NB], F32),
            "y": dram(p + "y", [DK, 128, cap], FP16, out=True),
        }
        slots.append((j, cap, cls, w))

    with tile.TileContext(nc) as tc:
        with (
            tc.tile_pool(name="persist", bufs=1) as persist,
            tc.tile_pool(name="wpool", bufs=6) as wpool,
            tc.tile_pool(name="w2pool", bufs=8) as w2pool,
            tc.tile_pool(name="work", bufs=2) as work,
            tc.tile_pool(name="outp", bufs=3) as outp,
            tc.tile_pool(name="ps", bufs=1, space="PSUM") as ps,
            tc.tile_pool(name="psy", bufs=3, space="PSUM") as psy,
        ):
            xts_t, bt_t, hb_t = {}, {}, {}
            for j, cap, cls, w in slots:
                dt = CLS[cls]["dt"]
                xts_t[j] = persist.tile([128, DK, cap], dt, tag=f"xt{j}",
                                        name=f"xt_s{j}")
                bt_t[j] = persist.tile([128, NB], F32, tag=f"bias{j}",
                                       name=f"bias_s{j}")
                hb_t[j] = persist.tile([128, IT, cap], dt, tag=f"h{j}",
                                       name=f"h_s{j}")

            def load_xt_bias(j):
                _, cap, _, w = slots[j]
                if j == 0:
                    half = DK // 2
                    xap = w["xt"].ap()
                    nc.gpsimd.dma_start(out=xts_t[j][:, :half],
                                        in_=xap[:, :half])
                    nc.gpsimd.dma_start(out=xts_t[j][:, half:],
                                        in_=xap[:, half:])
                else:
                    nc.gpsimd.dma_start(out=xts_t[j], in_=w["xt"].ap())
                nc.gpsimd.dma_start(out=bt_t[j], in_=w["bias"].ap())

            load_xt_bias(0)

            def wf_load(j, it):
                _, _, cls, w = slots[j]
                dt = CLS[cls]["dt"]
                wt = wpool.tile([128, 4 * DK, 128], dt, tag="wf",
                                name=f"wf_s{j}_{it}")
                if j == 0 and it <= 2:
                    for wi in range(4):
                        nc.sync.dma_start(
                            out=wt[:, wi * DK:(wi + 1) * DK, :],
                            in_=w["wf"][it][:, wi * DK:(wi + 1) * DK, :])
                else:
                    nc.sync.dma_start(out=wt, in_=w["wf"][it])
                return wt

            wf_pre = {}
            for j, cap, cls, w in slots:
                c = CLS[cls]
                s1 = c["SX"] * c["SW"]
                dbl = c["dbl"]
                grp = _groups(cap)
                xts, bt, hb = xts_t[j], bt_t[j], hb_t[j]
                w2_pre = {}

                def prefetch_w2(dk, j=j, cls=cls, w=w, w2_pre=w2_pre):
                    w2t = w2pool.tile([128, IT, 128], CLS[cls]["dt"], tag="w2",
                                      name=f"w2_s{j}_{dk}")
                    nc.sync.dma_start(out=w2t, in_=w["w2"][dk])
                    w2_pre[dk] = w2t

                # ---- first GEMM + swiglu: h[it, tok] ----
                for it in range(IT):
                    wt = wf_pre.pop((j, it), None)
                    if wt is None:
                        wt = wf_load(j, it)
                    if it == 8 and j + 1 < len(slots):
                        load_xt_bias(j + 1)
                    # all 8 w2 tiles land during GEMM1 so GEMM2 issues no
                    # Sync-queue loads behind the next slot's wf prefetch
                    W2_AT = {5: 0, 6: 1, 7: 2, 8: 3, 9: 4, 11: 5, 13: 6, 15: 7}
                    if it in W2_AT:
                        prefetch_w2(W2_AT[it])
                    for goff, gsz in grp:
                        accs = []
                        for wi in range(4):
                            acc = ps.tile([128, 512], F32, tag=f"acc{wi}",
                                          name=f"acc{wi}_s{j}_{it}_{goff}")
                            if dbl:
                                for p2 in range(DK // 2):
                                    nc.tensor.matmul(
                                        acc[:, :gsz],
                                        wt[:, wi * DK + 2 * p2:
                                           wi * DK + 2 * p2 + 2, :],
                                        xts[:, 2 * p2:2 * p2 + 2,
                                            goff:goff + gsz],
                                        start=(p2 == 0),
                                        stop=(p2 == DK // 2 - 1),
                                        perf_mode=mybir.MatmulPerfMode.DoubleRow)
                            else:
                                for dk in range(DK):
                                    nc.tensor.matmul(
                                        acc[:, :gsz],
                                        wt[:, wi * DK + dk, :],
                                        xts[:, dk, goff:goff + gsz],
                                        start=(dk == 0), stop=(dk == DK - 1))
                            accs.append(acc)
                        A, Bm, C, Dm = accs
                        # Bp = v_e = B/s1 + b3e
                        Bp = work.tile([128, 512], F32, tag="Bp")
                        nc.scalar.activation(Bp[:, :gsz], Bm[:, :gsz],
                                             Act.Identity, scale=1.0 / s1,
                                             bias=bt[:, IT + it:IT + it + 1])
                        # G = (A + s1*b1e) * Bp = s1*g
                        G = work.tile([128, 512], F32, tag="G")
                        nc.vector.scalar_tensor_tensor(
                            G[:, :gsz], A[:, :gsz], bt[:, it:it + 1],
                            Bp[:, :gsz], Alu.add, Alu.mult)
                        nc.vector.tensor_scalar_min(G[:, :gsz], G[:, :gsz],
                                                    LIMIT * s1)
                        # Sv = Silu(alpha*g) = alpha*g*sig(alpha*g)
                        Sv = work.tile([128, 512], F32, tag="Sv")
                        nc.scalar.activation(Sv[:, :gsz], G[:, :gsz],
                                             Act.Silu, scale=ALPHA / s1)
                        # Dp = (SH/s1)*v_o
                        Dp = work.tile([128, 512], F32, tag="Dp")
                        nc.scalar.activation(
                            Dp[:, :gsz], Dm[:, :gsz], Act.Identity,
                            scale=c["SH"] / (s1 * s1),
                            bias=bt[:, 3 * IT + it:3 * IT + it + 1])
                        # L = (C + s1*b1o) * Dp = SH*l
                        L = work.tile([128, 512], F32, tag="L")
                        nc.vector.scalar_tensor_tensor(
                            L[:, :gsz], C[:, :gsz],
                            bt[:, 2 * IT + it:2 * IT + it + 1],
                            Dp[:, :gsz], Alu.add, Alu.mult)
                        nc.vector.tensor_scalar(L[:, :gsz], L[:, :gsz],
                                                LIMIT * c["SH"],
                                                -LIMIT * c["SH"],
                                                Alu.min, Alu.max)
                        # hb = (L + SH)*Sv = SH*alpha*h
                        nc.vector.scalar_tensor_tensor(
                            hb[:, it, goff:goff + gsz],
                            L[:, :gsz], c["SH"], Sv[:, :gsz],
                            Alu.add, Alu.mult)

                if j + 1 < len(slots):
                    for it2 in range(5):
                        wf_pre[(j + 1, it2)] = wf_load(j + 1, it2)

                # ---- second GEMM: y[dk] = sum_it w2[dk,it].T @ h[it] ----
                y_scale = 1.0 / (c["SH"] * c["SW2"])
                for dk in range(DK):
                    if dk in w2_pre:
                        w2t = w2_pre.pop(dk)
                    else:
                        w2t = w2pool.tile([128, IT, 128], c["dt"], tag="w2",
                                          name=f"w2_s{j}_{dk}")
                        nc.sync.dma_start(out=w2t, in_=w["w2"][dk])
                    for goff, gsz in grp:
                        Y = psy.tile([128, 512], F32, tag="Y",
                                     name=f"Y_s{j}_{dk}_{goff}")
                        if dbl:
                            for p2 in range(IT // 2):
                                nc.tensor.matmul(
                                    Y[:, :gsz],
                                    w2t[:, 2 * p2:2 * p2 + 2, :],
                                    hb[:, 2 * p2:2 * p2 + 2, goff:goff + gsz],
                                    start=(p2 == 0), stop=(p2 == IT // 2 - 1),
                                    perf_mode=mybir.MatmulPerfMode.DoubleRow)
                        else:
                            for it in range(IT):
                                nc.tensor.matmul(
                                    Y[:, :gsz],
                                    w2t[:, it, :],
                                    hb[:, it, goff:goff + gsz],
                                    start=(it == 0), stop=(it == IT - 1))
                        yo = outp.tile([128, 512], FP16, tag="yo")
                        nc.scalar.activation(
                            yo[:, :gsz], Y[:, :gsz], Act.Identity,
                            scale=y_scale,
                            bias=bt[:, 4 * IT + dk:4 * IT + dk + 1])
                        nc.scalar.dma_start(
                            out=w["y"][dk, :, goff:goff + gsz],
                            in_=yo[:, :gsz])

    nc.finalize()
    return nc


# --------------------------------------------------------------------------
# host plan construction (shared with the accuracy sim)
# --------------------------------------------------------------------------

def _make_plan(xt, gate_w, gate_b):
    """Gate + routing + slot planning.  Returns a dict with everything the
    packer/combiner needs."""
    z = xt.astype(np.float64) @ np.asarray(gate_w, dtype=np.float64).T
    z -= z.max(axis=-1, keepdims=True)
    ez = np.exp(z)
    scores = ez / ez.sum(axis=-1, keepdims=True)
    biased = scores + np.asarray(gate_b, dtype=np.float64)
    top2 = np.argsort(-biased, axis=-1, kind="stable")[:, :TOPK]
    gate_wt = np.take_along_axis(scores, top2, axis=-1).astype(np.float32)

    tok_sorted, wt_sorted = [], []
    for e in range(E):
        sel = np.nonzero((top2 == e).any(axis=1))[0]
        we = np.where(top2[sel, 0] == e, gate_wt[sel, 0],
                      gate_wt[sel, 1]).astype(np.float32)
        order = np.argsort(we, kind="stable")
        tok_sorted.append(sel[order])
        wt_sorted.append(we[order])
    n_e = [len(s) for s in tok_sorted]

    plan = _plan_hybrid(n_e, wt_sorted)
    if plan is None:
        raise RuntimeError("no hybrid plan found")
    positions, asg = plan
    p8pos = sorted((i for i, (c, k) in enumerate(positions) if k == "p8"),
                   key=lambda i: -positions[i][0])
    p16pos = sorted((i for i, (c, k) in enumerate(positions) if k == "p16"),
                    key=lambda i: -positions[i][0])

    # token split per expert: bottom take8 go fp8, rest p16
    take8 = []
    for e in range(E):
        pc = sum(asg[e][i] * positions[i][0] for i in p16pos)
        take8.append(max(0, n_e[e] - pc))

    # pieces per position (larger caps filled first within each class)
    pieces = {i: [] for i in range(len(positions))}
    for e in range(E):
        lo = 0
        for i in p8pos:
            for _ in range(asg[e][i]):
                hi = min(lo + positions[i][0], take8[e])
                pieces[i].append((e, lo, hi))
                lo = hi
        assert lo >= take8[e]
        lo = take8[e]
        for i in p16pos:
            for _ in range(asg[e][i]):
                hi = min(lo + positions[i][0], n_e[e])
                pieces[i].append((e, lo, hi))
                lo = hi
        assert lo >= n_e[e]
    for i in pieces:
        assert len(pieces[i]) <= N_CORES
        while len(pieces[i]) < N_CORES:
            pieces[i].append((0, 0, 0))

    # device slot order: fp8 first (cheap fill), p16 middle, shared last
    entries = [(("pos", i), positions[i][0], "p8") for i in p8pos]
    entries += [(("pos", i), positions[i][0], "p16") for i in p16pos]
    entries.append(("sh", TS, "p16"))

    return dict(top2=top2, gate_wt=gate_wt, tok_sorted=tok_sorted,
                wt_sorted=wt_sorted, n_e=n_e, positions=positions,
                take8=take8, pieces=pieces, entries=entries)


# --------------------------------------------------------------------------
# entry point
# --------------------------------------------------------------------------

def kernel(x, gate_w, gate_b, w1, b1, w3, b3, w2, b2,
           sw1, sb1, sw3, sb3, sw2, sb2):
    x = np.asarray(x, dtype=np.float32)
    xt = x.reshape(T, D)

    plan = _make_plan(xt, gate_w, gate_b)
    entries = plan["entries"]
    pieces = plan["pieces"]
    tok_sorted, wt_sorted = plan["tok_sorted"], plan["wt_sorted"]

    epacks = {}

    def epack(e, cls):
        if (e, cls) not in epacks:
            epacks[(e, cls)] = _expert_pack(
                np.asarray(w1[e]), np.asarray(b1[e]), np.asarray(w3[e]),
                np.asarray(b3[e]), np.asarray(w2[e]), np.asarray(b2[e]), cls)
        return epacks[(e, cls)]

    spack = _expert_pack(np.asarray(sw1), np.asarray(sb1), np.asarray(sw3),
                         np.asarray(sb3), np.asarray(sw2), np.asarray(sb2),
                         "p16")

    in_maps = []
    for cc in range(N_CORES):
        m = {}
        for s, (kind, cap, cls) in enumerate(entries):
            if kind == "sh":
                m[f"s{s}xt"] = _xt_pack(xt[cc * TS:(cc + 1) * TS], TS, cls)
                pk = spack
            else:
                e, lo, hi = pieces[kind[1]][cc]
                m[f"s{s}xt"] = _xt_pack(xt[tok_sorted[e][lo:hi]], cap, cls)
                pk = epack(e, cls)
            for kk, v in pk.items():
                m[f"s{s}{kk}"] = v
        in_maps.append(m)

    slot_desc = tuple((cap, cls) for _, cap, cls in entries)
    if slot_desc not in _kernel_cache:
        _kernel_cache[slot_desc] = _build(slot_desc)
    nc = _kernel_cache[slot_desc]
    res = run_bass_kernel_spmd(nc, in_maps, list(range(N_CORES)))

    # ---- combine ----
    out = np.zeros((T, D), dtype=np.float32)
    for cc in range(N_CORES):
        for s, (kind, cap, cls) in enumerate(entries):
            yc = res.results[cc][f"s{s}y"].astype(np.float32)
            yc = yc.reshape(D, cap)
            if kind == "sh":
                out[cc * TS:(cc + 1) * TS] += yc.T
            else:
                e, lo, hi = pieces[kind[1]][cc]
                if hi <= lo:
                    continue
                idx = tok_sorted[e][lo:hi]
                out[idx] += wt_sorted[e][lo:hi][:, None] * yc.T[:hi - lo]
    return out.reshape(B, S, D)


# revision 15
# speedup vs baseline: 1.2304x; 1.0134x over previous
"""MoE (top-2 of 8 experts + shared expert) Trainium2 kernel, expert-parallel
across 8 NeuronCores, hybrid fp16/fp8 precision.

Strategy (upgrade over the 445us bf16 baseline):
  - Host: gate in float64 numpy; tokens dispatched by routing index.
  - Precision hybrid: each routed expert-visit's error contribution to the
    final output is attenuated by its gate combine weight (mean ~0.24), while
    the shared expert enters with weight 1.  So the lowest-weight ~2/3 of
    routed visits (chosen by a global sum-w^2 error budget FRAC) run fully in
    fp8 e4m3 with DoubleRow matmuls (2x PE rate, measured 1.92x), and the
    high-weight rest + the shared expert run in fp16 (same PE rate as bf16,
    4 more mantissa bits -> smaller base error).
  - Quantization scales (w x32, x x8 for fp8, h x2) keep values out of fp8/
    fp16 denormals and are folded into ACT scale/bias constants -- zero extra
    device ops vs the baseline swiglu (5 DVE + 3 ACT per i-tile).
  - Per-core slots: [shared 512 f16] + p16 routed slot(s) (caps from a DP
    cover of the residual per-expert counts) + one fp8 slot (cap CF, one
    expert per core, so its weights stream once).  Planner minimizes
    sum(caps16) + 0.52*CF subject to the error budget.
  - Scheduling: largest-compute slot first; x/bias loads on the GpSimd queue,
    y writebacks on the Scalar queue, weight loads on Sync; next-slot x at
    it==8, 5 wf tiles prefetched ahead of each slot's GEMM2, w2 prefetched
    at odd i-tiles.
"""
import sys

sys.path.insert(0, "/opt/trn_rl_repo")

import itertools
import os

import ml_dtypes
import numpy as np

import concourse.bacc as bacc_mod
import concourse.tile as tile
from concourse import mybir
from concourse.bass_utils import run_bass_kernel_spmd

F32 = mybir.dt.float32
FP16 = mybir.dt.float16
FP8 = mybir.dt.float8e4
NP_F8 = ml_dtypes.float8_e4m3
Alu = mybir.AluOpType
Act = mybir.ActivationFunctionType

ALPHA = 1.702
LIMIT = 7.0
TOPK = 2
D, I, E = 1024, 2048, 8
B, S = 2, 2048
T = B * S
DK = D // 128          # 8 d-tiles
IT = I // 128          # 16 i-tiles
TS = 512               # shared-expert tokens per core (T / 8)
N_CORES = 8
NB = 4 * IT + DK       # bias-pack columns

# fraction of routed sum-w^2 allowed into fp8 (error budget)
FRAC = float(os.environ.get("MOE_FRAC", "0.50"))
FP8_COST = 0.52        # measured fp8 PE cost per token vs fp16
DMA16 = 302            # p16 slot weight-stream floor, token units (21MB @ ~260GB/s)
DMA8 = 151             # fp8 slot weight-stream floor (10.5MB @ ~260GB/s)
SLOT_OH = 16           # per-slot pipeline overhead, token units

# per-class constants: s1 = SX*SW is the GEMM1 psum scale
CLS = {
    "p16": dict(dt=FP16, npdt=np.float16, SX=1.0, SW=32.0, SH=1.0, SW2=32.0,
                dbl=False),
    "p8": dict(dt=FP8, npdt=NP_F8, SX=8.0, SW=32.0, SH=2.0, SW2=32.0,
               dbl=True),
}

_kernel_cache = {}


# --------------------------------------------------------------------------
# host-side packing
# --------------------------------------------------------------------------

def _q(a, cls):
    if cls == "p8":
        return np.clip(a, -240.0, 240.0).astype(NP_F8)
    return a.astype(np.float16)


def _tile13(w):
    """[D, I] -> [IT, 128(k), DK, 128(m)]."""
    return w.reshape(DK, 128, IT, 128).transpose(2, 1, 0, 3)


def _expert_pack(w1, b1, w3, b3, w2, b2, cls):
    c = CLS[cls]
    s1 = c["SX"] * c["SW"]
    wf = np.stack([_tile13(w1[:, 0::2]), _tile13(w3[:, 0::2]),
                   _tile13(w1[:, 1::2]), _tile13(w3[:, 1::2])], axis=2)
    wf = np.ascontiguousarray(wf.reshape(IT, 128, 4 * DK, 128)) * c["SW"]
    w2t = (w2 * (c["SW2"] / ALPHA)).reshape(IT, 128, DK, 128)
    w2t = np.ascontiguousarray(w2t.transpose(2, 1, 0, 3))  # [DK,128,IT,128]
    bias = np.concatenate([
        s1 * b1[0::2].reshape(IT, 128).T,
        b3[0::2].reshape(IT, 128).T,
        s1 * b1[1::2].reshape(IT, 128).T,
        (c["SH"] / s1) * b3[1::2].reshape(IT, 128).T,
        b2.reshape(DK, 128).T,
    ], axis=1)
    return {
        "wf": _q(wf, cls),
        "w2": _q(w2t, cls),
        "bias": np.ascontiguousarray(bias, dtype=np.float32),
    }


def _xt_pack(xsub, cap, cls):
    """[n, D] tokens -> zero-padded [128, DK, cap] transposed layout."""
    c = CLS[cls]
    n = xsub.shape[0]
    xt = np.zeros((D, cap), dtype=np.float32)
    xt[:, :n] = (c["SX"] * xsub).T
    xt = np.ascontiguousarray(xt.reshape(DK, 128, cap).transpose(1, 0, 2))
    return _q(xt, cls)


# --------------------------------------------------------------------------
# planning
# --------------------------------------------------------------------------

def _cover(caps, counts, ninst=8):
    """Assign instances of each cap to experts covering counts."""
    k = len(caps)
    per = []
    for cnt in counts:
        out = []
        for combo in itertools.product(range(ninst + 1), repeat=k):
            cap = sum(n * c for n, c in zip(combo, caps))
            if cap >= cnt:
                out.append((cap - cnt, combo))
        if not out:
            return None
        out.sort()
        per.append([c for _, c in out[:64]])
    memo = {}

    def dp(i, used):
        if i == len(counts):
            return []
        key = (i, used)
        if key in memo:
            return memo[key]
        res = None
        for combo in per[i]:
            nu = tuple(u + n for u, n in zip(used, combo))
            if any(u > ninst for u in nu):
                continue
            sub = dp(i + 1, nu)
            if sub is not None:
                res = [combo] + sub
                break
        memo[key] = res
        return res

    return dp(0, (0,) * k)


def _slot_cost(cap, cls):
    if cls == "p8":
        return max(FP8_COST * cap, DMA8) + SLOT_OH
    return max(cap, DMA16) + SLOT_OH


def _expert_combos(positions, ne, cume):
    """Instance-count combos (one tuple per position) covering ne tokens,
    p16-first fill; value = fp8 sum-w^2."""
    i16 = [i for i, (c, k) in enumerate(positions) if k == "p16"]
    i8s = [i for i, (c, k) in enumerate(positions) if k == "p8"]
    out = []
    seen = set()
    ranges = [range(9)] * len(i16)
    for c16 in itertools.product(*ranges):
        pcap = sum(n * positions[i][0] for n, i in zip(c16, i16))
        f8 = max(0, ne - pcap)
        covers = []
        if f8 == 0:
            covers.append([0] * len(i8s))
        elif len(i8s) == 1:
            j = -(-f8 // positions[i8s[0]][0])
            if j <= 8:
                covers.append([j])
        elif len(i8s) == 2:
            ca, cb = positions[i8s[0]][0], positions[i8s[1]][0]
            for j1 in range(9):
                j2 = -(-max(0, f8 - j1 * ca) // cb)
                if j2 <= 8:
                    covers.append([j1, j2])
                    if j2 == 0:
                        break
        for cov in covers:
            ix = [0] * len(positions)
            for n, i in zip(c16, i16):
                ix[i] = n
            for n, i in zip(cov, i8s):
                ix[i] = n
            ixt = tuple(ix)
            if ixt in seen:
                continue
            seen.add(ixt)
            out.append((ixt, float(cume[f8])))
    return out


def _assign_np(positions, n_e, cum, budget):
    """Vectorized DP over instance-usage states; returns per-expert combo
    tuples or None."""
    P = len(positions)
    shape = (9,) * P
    INF = np.inf
    val = np.full(shape, INF)
    val[(0,) * P] = 0.0
    order = sorted(range(E), key=lambda e: -n_e[e])
    trace = []
    for e in order:
        combos = _expert_combos(positions, n_e[e], cum[e])
        if not combos:
            return None
        nv = np.full(shape, INF)
        pidx = np.full(shape, -1, dtype=np.int32)
        for ci, (ix, w2v) in enumerate(combos):
            src = val[tuple(slice(0, 9 - i) for i in ix)] + w2v
            dst = nv[tuple(slice(i, 9) for i in ix)]
            pv = pidx[tuple(slice(i, 9) for i in ix)]
            m = src < dst
            dst[m] = src[m]
            pv[m] = ci
        nv[nv > budget] = INF
        if not np.isfinite(nv).any():
            return None
        trace.append((e, combos, pidx))
        val = nv
    state = np.unravel_index(np.argmin(val), shape)
    if not np.isfinite(val[state]):
        return None
    out = [None] * E
    for e, combos, pidx in reversed(trace):
        ci = int(pidx[state])
        ix = combos[ci][0]
        out[e] = ix
        state = tuple(s - i for s, i in zip(state, ix))
    return out


def _plan_hybrid(n_e, wt_sorted):
    """Cost-ordered search over slot structures with DP budget feasibility.
    Returns (positions, asg): positions = [(cap, cls)...], asg[e] = instance
    counts per position."""
    cum = [np.concatenate([[0.0], np.cumsum(w.astype(np.float64) ** 2)])
           for w in wt_sorted]
    budget = FRAC * sum(float(c[-1]) for c in cum)
    tot = sum(n_e)

    f_grid = list(range(320, 897, 32))
    f2_grid = [0] + list(range(192, 897, 32))
    c_grid = list(range(224, 513, 16))
    cands = {}

    def add(poss):
        poss = tuple(sorted((p for p in poss if p[0] > 0),
                            key=lambda p: (p[1], -p[0])))
        if not poss or poss in cands:
            return
        if 8 * sum(c for c, _ in poss) < tot:
            return
        cands[poss] = sum(_slot_cost(c, k) for c, k in poss)

    for f1 in f_grid:
        for c1 in c_grid:
            add([(f1, "p8"), (c1, "p16")])
            for f2 in f2_grid:
                if f2 <= f1:
                    add([(f1, "p8"), (f2, "p8"), (c1, "p16")])
            for c2 in range(224, c1 + 1, 16):
                add([(f1, "p8"), (c1, "p16"), (c2, "p16")])
    for c1 in range(224, 513, 32):
        for c2 in range(224, c1 + 1, 32):
            add([(c1, "p16"), (c2, "p16")])
            for c3 in range(224, c2 + 1, 32):
                add([(c1, "p16"), (c2, "p16"), (c3, "p16")])

    for poss in sorted(cands, key=lambda p: cands[p]):
        asg = _assign_np(list(poss), n_e, cum, budget)
        if asg is not None:
            return list(poss), asg
    return None


# --------------------------------------------------------------------------
# device kernel
# --------------------------------------------------------------------------

def _groups(cap):
    gs = [512] * (cap // 512)
    if cap % 512:
        gs.append(cap % 512)
    offs = np.cumsum([0] + gs)[:-1]
    return list(zip(offs, gs))


def _build(slot_desc):
    """slot_desc: tuple of (cap, cls) in device order."""
    nc = bacc_mod.Bacc("TRN2")

    def dram(name, shape, dtype, out=False):
        return nc.declare_dram_parameter(name, list(shape), dtype, isOutput=out)

    slots = []
    for j, (cap, cls) in enumerate(slot_desc):
        p = f"s{j}"
        dt = CLS[cls]["dt"]
        w = {
            "xt": dram(p + "xt", [128, DK, cap], dt),
            "wf": dram(p + "wf", [IT, 128, 4 * DK, 128], dt),
            "w2": dram(p + "w2", [DK, 128, IT, 128], dt),
            "bias": dram(p + "bias", [128, NB], F32),
            "y": dram(p + "y", [DK, 128, cap], FP16, out=True),
        }
        slots.append((j, cap, cls, w))

    with tile.TileContext(nc) as tc:
        with (
            tc.tile_pool(name="persist", bufs=1) as persist,
            tc.tile_pool(name="wpool", bufs=7) as wpool,
            tc.tile_pool(name="w2pool", bufs=8) as w2pool,
            tc.tile_pool(name="work", bufs=2) as work,
            tc.tile_pool(name="outp", bufs=3) as outp,
            tc.tile_pool(name="ps", bufs=1, space="PSUM") as ps,
            tc.tile_pool(name="psy", bufs=3, space="PSUM") as psy,
        ):
            xts_t, bt_t, hb_t = {}, {}, {}
            for j, cap, cls, w in slots:
                dt = CLS[cls]["dt"]
                xts_t[j] = persist.tile([128, DK, cap], dt, tag=f"xt{j}",
                                        name=f"xt_s{j}")
                bt_t[j] = persist.tile([128, NB], F32, tag=f"bias{j}",
                                       name=f"bias_s{j}")
                hb_t[j] = persist.tile([128, IT, cap], dt, tag=f"h{j}",
                                       name=f"h_s{j}")

            def load_xt_bias(j):
                _, cap, _, w = slots[j]
                if j == 0:
                    half = DK // 2
                    xap = w["xt"].ap()
                    nc.gpsimd.dma_start(out=xts_t[j][:, :half],
                                        in_=xap[:, :half])
                    nc.gpsimd.dma_start(out=xts_t[j][:, half:],
                                        in_=xap[:, half:])
                else:
                    nc.gpsimd.dma_start(out=xts_t[j], in_=w["xt"].ap())
                nc.gpsimd.dma_start(out=bt_t[j], in_=w["bias"].ap())

            load_xt_bias(0)

            def wf_load(j, it):
                _, _, cls, w = slots[j]
                dt = CLS[cls]["dt"]
                wt = wpool.tile([128, 4 * DK, 128], dt, tag="wf",
                                name=f"wf_s{j}_{it}")
                if j == 0 and it <= 2:
                    for wi in range(4):
                        nc.sync.dma_start(
                            out=wt[:, wi * DK:(wi + 1) * DK, :],
                            in_=w["wf"][it][:, wi * DK:(wi + 1) * DK, :])
                else:
                    nc.sync.dma_start(out=wt, in_=w["wf"][it])
                return wt

            wf_pre = {}
            for j, cap, cls, w in slots:
                c = CLS[cls]
                s1 = c["SX"] * c["SW"]
                dbl = c["dbl"]
                grp = _groups(cap)
                xts, bt, hb = xts_t[j], bt_t[j], hb_t[j]
                w2_pre = {}

                def prefetch_w2(dk, j=j, cls=cls, w=w, w2_pre=w2_pre):
                    w2t = w2pool.tile([128, IT, 128], CLS[cls]["dt"], tag="w2",
                                      name=f"w2_s{j}_{dk}")
                    nc.sync.dma_start(out=w2t, in_=w["w2"][dk])
                    w2_pre[dk] = w2t

                # ---- first GEMM + swiglu: h[it, tok] ----
                for it in range(IT):
                    wt = wf_pre.pop((j, it), None)
                    if wt is None:
                        wt = wf_load(j, it)
                    if it == 8 and j + 1 < len(slots):
                        load_xt_bias(j + 1)
                    # all 8 w2 tiles land during GEMM1 so GEMM2 issues no
                    # Sync-queue loads behind the next slot's wf prefetch
                    W2_AT = {5: 0, 6: 1, 7: 2, 8: 3, 9: 4, 11: 5, 13: 6, 15: 7}
                    if it in W2_AT:
                        prefetch_w2(W2_AT[it])
                    for goff, gsz in grp:
                        accs = []
                        for wi in range(4):
                            acc = ps.tile([128, 512], F32, tag=f"acc{wi}",
                                          name=f"acc{wi}_s{j}_{it}_{goff}")
                            if dbl:
                                for p2 in range(DK // 2):
                                    nc.tensor.matmul(
                                        acc[:, :gsz],
                                        wt[:, wi * DK + 2 * p2:
                                           wi * DK + 2 * p2 + 2, :],
                                        xts[:, 2 * p2:2 * p2 + 2,
                                            goff:goff + gsz],
                                        start=(p2 == 0),
                                        stop=(p2 == DK // 2 - 1),
                                        perf_mode=mybir.MatmulPerfMode.DoubleRow)
                            else:
                                for dk in range(DK):
                                    nc.tensor.matmul(
                                        acc[:, :gsz],
                                        wt[:, wi * DK + dk, :],
                                        xts[:, dk, goff:goff + gsz],
                                        start=(dk == 0), stop=(dk == DK - 1))
                            accs.append(acc)
                        A, Bm, C, Dm = accs
                        # Bp = v_e = B/s1 + b3e
                        Bp = work.tile([128, 512], F32, tag="Bp")
                        nc.scalar.activation(Bp[:, :gsz], Bm[:, :gsz],
                                             Act.Identity, scale=1.0 / s1,
                                             bias=bt[:, IT + it:IT + it + 1])
                        # G = (A + s1*b1e) * Bp = s1*g
                        G = work.tile([128, 512], F32, tag="G")
                        nc.vector.scalar_tensor_tensor(
                            G[:, :gsz], A[:, :gsz], bt[:, it:it + 1],
                            Bp[:, :gsz], Alu.add, Alu.mult)
                        nc.vector.tensor_scalar_min(G[:, :gsz], G[:, :gsz],
                                                    LIMIT * s1)
                        # Sv = Silu(alpha*g) = alpha*g*sig(alpha*g)
                        Sv = work.tile([128, 512], F32, tag="Sv")
                        nc.scalar.activation(Sv[:, :gsz], G[:, :gsz],
                                             Act.Silu, scale=ALPHA / s1)
                        # Dp = (SH/s1)*v_o
                        Dp = work.tile([128, 512], F32, tag="Dp")
                        nc.scalar.activation(
                            Dp[:, :gsz], Dm[:, :gsz], Act.Identity,
                            scale=c["SH"] / (s1 * s1),
                            bias=bt[:, 3 * IT + it:3 * IT + it + 1])
                        # L = (C + s1*b1o) * Dp = SH*l
                        L = work.tile([128, 512], F32, tag="L")
                        nc.vector.scalar_tensor_tensor(
                            L[:, :gsz], C[:, :gsz],
                            bt[:, 2 * IT + it:2 * IT + it + 1],
                            Dp[:, :gsz], Alu.add, Alu.mult)
                        nc.vector.tensor_scalar(L[:, :gsz], L[:, :gsz],
                                                LIMIT * c["SH"],
                                                -LIMIT * c["SH"],
                                                Alu.min, Alu.max)
                        # hb = (L + SH)*Sv = SH*alpha*h
                        nc.vector.scalar_tensor_tensor(
                            hb[:, it, goff:goff + gsz],
                            L[:, :gsz], c["SH"], Sv[:, :gsz],
                            Alu.add, Alu.mult)

                if j + 1 < len(slots):
                    for it2 in range(6):
                        wf_pre[(j + 1, it2)] = wf_load(j + 1, it2)

                # ---- second GEMM: y[dk] = sum_it w2[dk,it].T @ h[it] ----
                y_scale = 1.0 / (c["SH"] * c["SW2"])
                for dk in range(DK):
                    if dk in w2_pre:
                        w2t = w2_pre.pop(dk)
                    else:
                        w2t = w2pool.tile([128, IT, 128], c["dt"], tag="w2",
                                          name=f"w2_s{j}_{dk}")
                        nc.sync.dma_start(out=w2t, in_=w["w2"][dk])
                    for goff, gsz in grp:
                        Y = psy.tile([128, 512], F32, tag="Y",
                                     name=f"Y_s{j}_{dk}_{goff}")
                        if dbl:
                            for p2 in range(IT // 2):
                                nc.tensor.matmul(
                                    Y[:, :gsz],
                                    w2t[:, 2 * p2:2 * p2 + 2, :],
                                    hb[:, 2 * p2:2 * p2 + 2, goff:goff + gsz],
                                    start=(p2 == 0), stop=(p2 == IT // 2 - 1),
                                    perf_mode=mybir.MatmulPerfMode.DoubleRow)
                        else:
                            for it in range(IT):
                                nc.tensor.matmul(
                                    Y[:, :gsz],
                                    w2t[:, it, :],
                                    hb[:, it, goff:goff + gsz],
                                    start=(it == 0), stop=(it == IT - 1))
                        yo = outp.tile([128, 512], FP16, tag="yo")
                        nc.scalar.activation(
                            yo[:, :gsz], Y[:, :gsz], Act.Identity,
                            scale=y_scale,
                            bias=bt[:, 4 * IT + dk:4 * IT + dk + 1])
                        nc.scalar.dma_start(
                            out=w["y"][dk, :, goff:goff + gsz],
                            in_=yo[:, :gsz])

    nc.finalize()
    return nc


# --------------------------------------------------------------------------
# host plan construction (shared with the accuracy sim)
# --------------------------------------------------------------------------

def _make_plan(xt, gate_w, gate_b):
    """Gate + routing + slot planning.  Returns a dict with everything the
    packer/combiner needs."""
    z = xt.astype(np.float64) @ np.asarray(gate_w, dtype=np.float64).T
    z -= z.max(axis=-1, keepdims=True)
    ez = np.exp(z)
    scores = ez / ez.sum(axis=-1, keepdims=True)
    biased = scores + np.asarray(gate_b, dtype=np.float64)
    top2 = np.argsort(-biased, axis=-1, kind="stable")[:, :TOPK]
    gate_wt = np.take_along_axis(scores, top2, axis=-1).astype(np.float32)

    tok_sorted, wt_sorted = [], []
    for e in range(E):
        sel = np.nonzero((top2 == e).any(axis=1))[0]
        we = np.where(top2[sel, 0] == e, gate_wt[sel, 0],
                      gate_wt[sel, 1]).astype(np.float32)
        order = np.argsort(we, kind="stable")
        tok_sorted.append(sel[order])
        wt_sorted.append(we[order])
    n_e = [len(s) for s in tok_sorted]

    plan = _plan_hybrid(n_e, wt_sorted)
    if plan is None:
        raise RuntimeError("no hybrid plan found")
    positions, asg = plan
    p8pos = sorted((i for i, (c, k) in enumerate(positions) if k == "p8"),
                   key=lambda i: -positions[i][0])
    p16pos = sorted((i for i, (c, k) in enumerate(positions) if k == "p16"),
                    key=lambda i: -positions[i][0])

    # token split per expert: bottom take8 go fp8, rest p16
    take8 = []
    for e in range(E):
        pc = sum(asg[e][i] * positions[i][0] for i in p16pos)
        take8.append(max(0, n_e[e] - pc))

    # pieces per position (larger caps filled first within each class)
    pieces = {i: [] for i in range(len(positions))}
    for e in range(E):
        lo = 0
        for i in p8pos:
            for _ in range(asg[e][i]):
                hi = min(lo + positions[i][0], take8[e])
                pieces[i].append((e, lo, hi))
                lo = hi
        assert lo >= take8[e]
        lo = take8[e]
        for i in p16pos:
            for _ in range(asg[e][i]):
                hi = min(lo + positions[i][0], n_e[e])
                pieces[i].append((e, lo, hi))
                lo = hi
        assert lo >= n_e[e]
    for i in pieces:
        assert len(pieces[i]) <= N_CORES
        while len(pieces[i]) < N_CORES:
            pieces[i].append((0, 0, 0))

    # device slot order: fp8 first (cheap fill), p16 middle, shared last
    entries = [(("pos", i), positions[i][0], "p8") for i in p8pos]
    entries += [(("pos", i), positions[i][0], "p16") for i in p16pos]
    entries.append(("sh", TS, "p16"))

    return dict(top2=top2, gate_wt=gate_wt, tok_sorted=tok_sorted,
                wt_sorted=wt_sorted, n_e=n_e, positions=positions,
                take8=take8, pieces=pieces, entries=entries)


# --------------------------------------------------------------------------
# entry point
# --------------------------------------------------------------------------

def kernel(x, gate_w, gate_b, w1, b1, w3, b3, w2, b2,
           sw1, sb1, sw3, sb3, sw2, sb2):
    x = np.asarray(x, dtype=np.float32)
    xt = x.reshape(T, D)

    plan = _make_plan(xt, gate_w, gate_b)
    entries = plan["entries"]
    pieces = plan["pieces"]
    tok_sorted, wt_sorted = plan["tok_sorted"], plan["wt_sorted"]

    epacks = {}

    def epack(e, cls):
        if (e, cls) not in epacks:
            epacks[(e, cls)] = _expert_pack(
                np.asarray(w1[e]), np.asarray(b1[e]), np.asarray(w3[e]),
                np.asarray(b3[e]), np.asarray(w2[e]), np.asarray(b2[e]), cls)
        return epacks[(e, cls)]

    spack = _expert_pack(np.asarray(sw1), np.asarray(sb1), np.asarray(sw3),
                         np.asarray(sb3), np.asarray(sw2), np.asarray(sb2),
                         "p16")

    in_maps = []
    for cc in range(N_CORES):
        m = {}
        for s, (kind, cap, cls) in enumerate(entries):
            if kind == "sh":
                m[f"s{s}xt"] = _xt_pack(xt[cc * TS:(cc + 1) * TS], TS, cls)
                pk = spack
            else:
                e, lo, hi = pieces[kind[1]][cc]
                m[f"s{s}xt"] = _xt_pack(xt[tok_sorted[e][lo:hi]], cap, cls)
                pk = epack(e, cls)
            for kk, v in pk.items():
                m[f"s{s}{kk}"] = v
        in_maps.append(m)

    slot_desc = tuple((cap, cls) for _, cap, cls in entries)
    if slot_desc not in _kernel_cache:
        _kernel_cache[slot_desc] = _build(slot_desc)
    nc = _kernel_cache[slot_desc]
    res = run_bass_kernel_spmd(nc, in_maps, list(range(N_CORES)))

    # ---- combine ----
    out = np.zeros((T, D), dtype=np.float32)
    for cc in range(N_CORES):
        for s, (kind, cap, cls) in enumerate(entries):
            yc = res.results[cc][f"s{s}y"].astype(np.float32)
            yc = yc.reshape(D, cap)
            if kind == "sh":
                out[cc * TS:(cc + 1) * TS] += yc.T
            else:
                e, lo, hi = pieces[kind[1]][cc]
                if hi <= lo:
                    continue
                idx = tok_sorted[e][lo:hi]
                out[idx] += wt_sorted[e][lo:hi][:, None] * yc.T[:hi - lo]
    return out.reshape(B, S, D)
